# revision 18
# baseline (speedup 1.0000x reference)
"""Trainium2 Bass kernel for nn_Experiment6 (bi-mamba + MHA + FFN forecaster).

Sharding: data-parallel over batch (B=8) across 8 NeuronCores; all params
replicated. Inside each core: activations kept transposed [feature, time];
selective scan via DVE tensor_tensor_scan in n-major layout
[128 d-partitions, (n=16, t=512) free]; reverse-direction mamba handled with
reversed free-axis APs (no data reversal). Output depends only on positions
0,1 of the final sequence, so the last layer is pruned accordingly.
RevIN normalization and final rescale are host-side (exact fp32).
"""
import numpy as np

import concourse.bacc as bacc
import concourse.bass as bass
import concourse.tile as tile
from concourse import mybir
from concourse.bass_utils import run_bass_kernel_spmd

FP = mybir.dt.float32
BF = mybir.dt.bfloat16
AF = mybir.ActivationFunctionType
OP = mybir.AluOpType

L = 512
DM = 512
DS = 16
DF = 2048
DTR = 32
NH = 4
DH = 128
PRED = 96
EPS = 1e-5
NB = 4  # number of 128-partition blocks in DM


def _f(x):
    return np.ascontiguousarray(np.asarray(x, np.float32))


def _bf(x):
    import ml_dtypes
    return np.ascontiguousarray(np.asarray(x, np.float32).astype(ml_dtypes.bfloat16))


def prep_host_inputs(inputs):
    """Returns (shared weight map, per-core x maps, per-core (mean, std))."""
    w = {}
    w["Wp"] = _bf(inputs["Wp"])                                # [2, 512]
    w["bp"] = _f(inputs["bp"])
    s = 1.0 / np.sqrt(DH)
    w["Wq"] = _bf(_f(inputs["Wq"]) * s)
    w["bq"] = _f(_f(inputs["bq"]) * s)
    w["Wk"] = _bf(inputs["Wk"])
    w["bk"] = _f(inputs["bk"])
    w["Wv"] = _bf(inputs["Wv"])
    w["Wo"] = _bf(inputs["Wo"])
    # fold v-bias through Wo, plus bi (the empty-input branch bias)
    bo2 = _f(inputs["bo"]) + _f(inputs["bi"]) + _f(inputs["Wo"]).T @ _f(inputs["bv"])
    w["bo2"] = _f(bo2)
    for li in range(2):
        for dd in range(2):
            tag = f"{li}{dd}"
            w["Win" + tag] = _bf(inputs["m_Win"][li, dd])       # [512, 1024]
            w["convw" + tag] = _f(inputs["m_convw"][li, dd])    # [512, 2]
            w["convb" + tag] = _f(inputs["m_convb"][li, dd])    # [512]
            w["Wx" + tag] = _bf(inputs["m_Wx"][li, dd])         # [512, 64]
            w["Wdt" + tag] = _bf(inputs["m_Wdt"][li, dd])       # [32, 512]
            w["bdt" + tag] = _f(inputs["m_bdt"][li, dd])        # [512]
            w["Wout" + tag] = _bf(inputs["m_Wout"][li, dd])     # [512, 512]
    for li in range(2):
        w[f"ffW1_{li}"] = _bf(inputs["ff_W1"][li])              # [512, 2048]
        w[f"ffb1_{li}"] = _f(inputs["ff_b1"][li])
        w[f"ffW2_{li}"] = _bf(inputs["ff_W2"][li])              # [2048, 512]
        w[f"ffb2_{li}"] = _f(inputs["ff_b2"][li])
    w["projW"] = _bf(inputs["proj_W"])                          # [512, 96]
    w["projb"] = _f(inputs["proj_b"])

    x_enc = _f(inputs["x_enc"])                                 # [8, 512, 2]
    means = x_enc.mean(1, keepdims=True)                        # [8,1,2]
    xc = x_enc - means
    stdev = np.sqrt(xc.var(axis=1, keepdims=True) + 1e-5)
    xn = xc / stdev
    xts = [np.ascontiguousarray(xn[b].T) for b in range(8)]     # [2,512] each
    return w, xts, means[:, 0, :], stdev[:, 0, :]


def rev3(t):
    """Flat reversed AP over a contiguous [128, 16, 512] n-major tile: iterates
    (n desc, t desc) so each n-chain runs t-descending; block transitions are
    cut by the a=0 mask at t=511."""
    el = t.ap[-1][0]
    ntot = t.shape[1] * t.shape[2]
    return bass.AP(tensor=t.tensor, offset=t.offset + (ntot - 1) * el,
                   ap=[t.ap[0], [-el, ntot]])


def flat2(t, ntot):
    el = t.ap[-1][0]
    return bass.AP(tensor=t.tensor, offset=t.offset, ap=[t.ap[0], [el, ntot]])


def build_program():
    nc = bacc.Bacc()
    P = {}

    def par(name, shape, dt):
        P[name] = nc.declare_dram_parameter(name, list(shape), dt, isOutput=False)
        return P[name]

    par("xT", (2, L), FP)
    par("Wp", (2, DM), BF); par("bp", (DM,), FP)
    for nm in ("Wq", "Wk", "Wv", "Wo"):
        par(nm, (DM, DM), BF)
    par("bq", (DM,), FP); par("bk", (DM,), FP); par("bo2", (DM,), FP)
    for li in range(2):
        for dd in range(2):
            tg = f"{li}{dd}"
            par("Win" + tg, (DM, 2 * DM), BF)
            par("convw" + tg, (DM, 2), FP)
            par("convb" + tg, (DM,), FP)
            par("Wx" + tg, (DM, DTR + 2 * DS), BF)
            par("Wdt" + tg, (DTR, DM), BF)
            par("bdt" + tg, (DM,), FP)
            par("Wout" + tg, (DM, DM), BF)
    for li in range(2):
        par(f"ffW1_{li}", (DM, DF), BF); par(f"ffb1_{li}", (DF,), FP)
        par(f"ffW2_{li}", (DF, DM), BF); par(f"ffb2_{li}", (DM,), FP)
    par("projW", (DM, PRED), BF); par("projb", (PRED,), FP)
    out_d = nc.declare_dram_parameter("out", [PRED, 2], FP, isOutput=True)

    with tile.TileContext(nc) as tc:
        import contextlib
        ctx = contextlib.ExitStack()
        with ctx:
            sing = ctx.enter_context(tc.tile_pool(name="sing", bufs=1))
            scr = ctx.enter_context(tc.tile_pool(name="scr", bufs=2))
            scr1 = ctx.enter_context(tc.tile_pool(name="scr1", bufs=1))
            wpool = ctx.enter_context(tc.tile_pool(name="wp", bufs=1))
            big = ctx.enter_context(tc.tile_pool(name="big", bufs=1))
            psum = ctx.enter_context(tc.tile_pool(name="ps", bufs=2, space="PSUM"))
            psacc = ctx.enter_context(tc.tile_pool(name="psacc", bufs=4, space="PSUM"))
            pss = ctx.enter_context(tc.tile_pool(name="pss", bufs=2, space="PSUM"))
            dram = ctx.enter_context(tc.tile_pool(name="dr", bufs=1, space="DRAM"))

            def vec(name, n=DM, dt=FP):
                """load a DRAM vector as NB [128,1] bias tiles"""
                ts = []
                for g in range(n // 128):
                    t = sing.tile([128, 1], dt, tag=f"v_{name}_{g}", name=f"v_{name}_{g}")
                    nc.sync.dma_start(out=t, in_=P[name][g * 128:(g + 1) * 128])
                    ts.append(t)
                return ts

            def wload(name, rows, cols, tag=None, dt=BF):
                """load weight [rows, cols] as rows//128 k-tiles"""
                ts = []
                nk = max(1, rows // 128)
                kr = rows // nk
                for k in range(nk):
                    t = wpool.tile([kr, cols], dt, tag=(tag or name) + f"_{k}")
                    nc.sync.dma_start(out=t, in_=P[name][k * kr:(k + 1) * kr, :])
                    ts.append(t)
                return ts

            ones_c = sing.tile([128, 1], FP)
            nc.vector.memset(ones_c, 1.0)
            ones_r = sing.tile([1, 128], FP)
            nc.vector.memset(ones_r, 1.0)
            eps_t = sing.tile([1, 1], FP)
            nc.vector.memset(eps_t, EPS)

            # ---- embed: ppT = Wp^T @ xT + bp ----
            xT = sing.tile([2, L], FP)
            nc.sync.dma_start(out=xT, in_=P["xT"][:, :])
            xTb = sing.tile([2, L], BF)
            nc.vector.tensor_copy(out=xTb, in_=xT)
            Wp_t = wload("Wp", 2, DM, tag="wp512x")  # [2, 512] single tile (rows<128)
            bp_t = vec("bp")
            pp_bf = [sing.tile([128, L], BF, tag=f"ppbf{g}", name=f"ppbf{g}") for g in range(NB)]
            for g in range(NB):
                ps = psum.tile([128, L], FP, tag="tr", name="tr")
                nc.tensor.matmul(ps, lhsT=Wp_t[0][:, g * 128:(g + 1) * 128],
                                 rhs=xTb, start=True, stop=True)
                nc.vector.tensor_scalar(out=pp_bf[g], in0=ps, scalar1=bp_t[g],
                                        scalar2=None, op0=OP.add)

            # ---- MHA ----
            def proj_T(wname, bias_ts, outdt=BF):
                """outT[do, t] = W^T @ pp (+bias): returns NB tiles"""
                Wt = wload(wname, DM, DM, tag="w512")
                outs = []
                for m in range(NB):
                    ps = psum.tile([128, L], FP, tag="tr", name="tr")
                    for k in range(NB):
                        nc.tensor.matmul(ps, lhsT=Wt[k][:, m * 128:(m + 1) * 128],
                                         rhs=pp_bf[k], start=(k == 0),
                                         stop=(k == NB - 1))
                    o = sing.tile([128, L], outdt, tag=f"{wname}_o{m}", name=f"{wname}_o{m}")
                    if bias_ts is None:
                        nc.scalar.copy(out=o, in_=ps)
                    else:
                        nc.vector.tensor_scalar(out=o, in0=ps, scalar1=bias_ts[m],
                                                scalar2=None, op0=OP.add)
                    outs.append(o)
                return outs

            qT = proj_T("Wq", vec("bq"))
            kT = proj_T("Wk", vec("bk"))
            # V in natural layout: V[t, d] = pp[t, :] @ Wv
            Wv_t = wload("Wv", DM, DM, tag="w512")
            Vn = []
            for m in range(NB):  # m indexes t-blocks
                ps = psum.tile([128, L], FP, tag="tr", name="tr")
                for k in range(NB):
                    nc.tensor.matmul(ps, lhsT=pp_bf[k][:, m * 128:(m + 1) * 128],
                                     rhs=Wv_t[k], start=(k == 0), stop=(k == NB - 1))
                o = sing.tile([128, L], BF, tag=f"vn{m}", name=f"vn{m}")
                nc.scalar.copy(out=o, in_=ps)
                Vn.append(o)

            oT = [sing.tile([128, L], BF, tag=f"oT{h}", name=f"oT{h}") for h in range(NH)]
            for h in range(NH):
                # ST[m, l] = K_h^T Q_h ; E = exp(ST); denom = ones^T E
                E_h = []
                dn = pss.tile([1, L], FP, tag="sm", name="sm")
                for mb in range(NB):
                    ps = psum.tile([128, L], FP, tag="tr", name="tr")
                    nc.tensor.matmul(ps, lhsT=kT[h][:, mb * 128:(mb + 1) * 128],
                                     rhs=qT[h], start=True, stop=True)
                    e = scr1.tile([128, L], BF, tag=f"eh{mb}", name=f"eh{mb}")
                    nc.scalar.activation(out=e, in_=ps, func=AF.Exp)
                    E_h.append(e)
                ob = scr.tile([1, 128], BF, tag="onesbf", name="onesbf")
                nc.vector.tensor_copy(out=ob, in_=ones_r)
                oc = scr.tile([128, 1], BF, tag="onescbf", name="onescbf")
                nc.vector.tensor_copy(out=oc, in_=ones_c)
                for mb in range(NB):
                    nc.tensor.matmul(dn, lhsT=oc, rhs=E_h[mb],
                                     start=(mb == 0), stop=(mb == NB - 1))
                rinv = scr.tile([1, L], FP, tag="rinv", name="rinv")
                nc.vector.reciprocal(out=rinv, in_=dn)
                rb = scr.tile([1, L], BF, tag="rb", name="rb")
                nc.vector.tensor_copy(out=rb, in_=rinv)
                rrep = psum.tile([128, L], FP, tag="tr", name="tr")
                nc.tensor.matmul(rrep, lhsT=ob, rhs=rb, start=True, stop=True)
                rrs = scr.tile([128, L], FP, tag="rrs", name="rrs")
                nc.scalar.copy(out=rrs, in_=rrep)
                # AV: OT_h = sum_m V[m, dh] E[m, l]
                av = psum.tile([128, L], FP, tag="tr", name="tr")
                for mb in range(NB):
                    nc.tensor.matmul(av, lhsT=Vn[mb][:, h * 128:(h + 1) * 128],
                                     rhs=E_h[mb], start=(mb == 0),
                                     stop=(mb == NB - 1))
                nc.vector.tensor_tensor(out=oT[h], in0=av, in1=rrs, op=OP.mult)

            bo2_t = vec("bo2")
            Wo_t = wload("Wo", DM, DM, tag="w512")
            hT = [sing.tile([128, L], FP, tag=f"hT{g}", name=f"hT{g}") for g in range(NB)]
            for m in range(NB):
                ps = psum.tile([128, L], FP, tag="tr", name="tr")
                for k in range(NB):
                    nc.tensor.matmul(ps, lhsT=Wo_t[k][:, m * 128:(m + 1) * 128],
                                     rhs=oT[k], start=(k == 0), stop=(k == NB - 1))
                nc.vector.tensor_scalar(out=hT[m], in0=ps, scalar1=bo2_t[m],
                                        scalar2=None, op0=OP.add)

            # ---- persistent mamba tiles ----
            NH2 = DS // 4
            A_blk = big.tile([128, NH2, L], BF, tag="Ablk", name="Ablk")
            dBu_blk = big.tile([128, NH2, L], BF, tag="dBublk", name="dBublk")
            B_rep = big.tile([128, NH2, L], BF, tag="Brep", name="Brep")
            C_rep = big.tile([128, NH2, L], BF, tag="Crep", name="Crep")
            dbl_dram = dram.tile([64, L], BF, tag="dbldram", name="dbldram")

            def emit_mamba(li, dd, h_bf, last):
                tg = f"{li}{dd}"
                rev = dd == 1
                Tn = 2 if (last and not rev) else L
                # Win matmuls: x-half always full T (rev) or Tn; z-half Tn2
                def win_half(co):
                    ts = []
                    for k in range(NB):
                        t = wpool.tile([128, DM], BF, tag=f"win_{k}",
                                       name=f"win_{k}")
                        nc.sync.dma_start(
                            out=t, in_=P["Win" + tg][k * 128:(k + 1) * 128,
                                                     co:co + DM])
                        ts.append(t)
                    return ts

                Win_t = win_half(0)
                Tx = L if not last or rev else 3
                xcpre = []
                for m in range(NB):
                    ps = psum.tile([128, L], FP, tag="tr", name="tr")
                    for k in range(NB):
                        nc.tensor.matmul(ps[:, 0:Tx],
                                         lhsT=Win_t[k][:, m * 128:(m + 1) * 128],
                                         rhs=h_bf[k][:, 0:Tx], start=(k == 0),
                                         stop=(k == NB - 1))
                    o = scr1.tile([128, L], FP, tag=f"xcpre{m}", name=f"xcpre{m}")
                    nc.scalar.copy(out=o[:, 0:Tx], in_=ps[:, 0:Tx])
                    xcpre.append(o)
                Tz = 2 if last else L
                Win_z = win_half(DM)
                zsil = []
                for m in range(NB):
                    ps = psum.tile([128, L], FP, tag="tr", name="tr")
                    for k in range(NB):
                        nc.tensor.matmul(
                            ps[:, 0:Tz],
                            lhsT=Win_z[k][:, m * 128:(m + 1) * 128],
                            rhs=h_bf[k][:, 0:Tz], start=(k == 0), stop=(k == NB - 1))
                    o = sing.tile([128, L], BF, tag=f"zsil{m}", name=f"zsil{m}")
                    nc.scalar.activation(out=o[:, 0:Tz], in_=ps[:, 0:Tz], func=AF.Silu)
                    zsil.append(o)

                convw = P["convw" + tg]
                w0 = [sing.tile([128, 1], FP, tag=f"w0_{g}", name=f"w0_{g}") for g in range(NB)]
                w1 = [sing.tile([128, 1], FP, tag=f"w1_{g}", name=f"w1_{g}") for g in range(NB)]
                for g in range(NB):
                    nc.sync.dma_start(out=w0[g],
                                      in_=convw[g * 128:(g + 1) * 128, 0:1])
                    nc.sync.dma_start(out=w1[g],
                                      in_=convw[g * 128:(g + 1) * 128, 1:2])
                cb_t = vec("convb" + tg)
                xcT = [sing.tile([128, L], BF, tag=f"xcT{g}", name=f"xcT{g}") for g in range(NB)]
                Tc = Tx if (last and not rev) else L
                for g in range(NB):
                    t1 = scr.tile([128, L], FP, tag="convt1", name="convt1")
                    nc.vector.tensor_scalar(out=t1[:, 0:Tc], in0=xcpre[g][:, 0:Tc],
                                            scalar1=w1[g], scalar2=cb_t[g],
                                            op0=OP.mult, op1=OP.add)
                    c2 = scr.tile([128, L], FP, tag="convt2", name="convt2")
                    if not rev:
                        nc.vector.scalar_tensor_tensor(
                            out=c2[:, 1:Tc], in0=xcpre[g][:, 0:Tc - 1],
                            scalar=w0[g], in1=t1[:, 1:Tc], op0=OP.mult, op1=OP.add)
                        nc.vector.tensor_copy(out=c2[:, 0:1], in_=t1[:, 0:1])
                    else:
                        nc.vector.scalar_tensor_tensor(
                            out=c2[:, 0:Tc - 1], in0=xcpre[g][:, 1:Tc],
                            scalar=w0[g], in1=t1[:, 0:Tc - 1], op0=OP.mult,
                            op1=OP.add)
                        nc.vector.tensor_copy(out=c2[:, Tc - 1:Tc],
                                              in_=t1[:, Tc - 1:Tc])
                    nc.scalar.activation(out=xcT[g][:, 0:Tn], in_=c2[:, 0:Tn],
                                         func=AF.Silu)

                # dbl = Wx^T @ xc  [64, Tn]
                Wx_t = wload("Wx" + tg, DM, 64, tag="wx")
                psd = pss.tile([64, L], FP, tag="sm", name="sm")
                for k in range(NB):
                    nc.tensor.matmul(psd[:, 0:Tn], lhsT=Wx_t[k],
                                     rhs=xcT[k][:, 0:Tn],
                                     start=(k == 0), stop=(k == NB - 1))
                dblT = scr.tile([64, L], FP, tag="dblT", name="dblT")
                nc.scalar.copy(out=dblT[:, 0:Tn], in_=psd[:, 0:Tn])
                dbl_bf = scr.tile([64, L], BF, tag="dblbf", name="dblbf")
                nc.vector.tensor_copy(out=dbl_bf[:, 0:Tn], in_=dblT[:, 0:Tn])
                nc.sync.dma_start(out=dbl_dram[:, 0:Tn], in_=dbl_bf[:, 0:Tn])
                dtraw = scr.tile([DTR, L], BF, tag="dtraw", name="dtraw")
                nc.vector.tensor_copy(out=dtraw[:, 0:Tn], in_=dblT[0:DTR, 0:Tn])

                # dt = softplus(Wdt^T @ dtraw + bdt)
                Wdt_t = wload("Wdt" + tg, DTR, DM, tag="wdt512")
                bdt_t = vec("bdt" + tg)
                dtT = [sing.tile([128, L], FP, tag=f"dtT{g}", name=f"dtT{g}") for g in range(NB)]
                duT = [sing.tile([128, L], BF, tag=f"duT{g}", name=f"duT{g}") for g in range(NB)]
                for g in range(NB):
                    ps = psum.tile([128, L], FP, tag="tr", name="tr")
                    nc.tensor.matmul(ps[:, 0:Tn],
                                     lhsT=Wdt_t[0][:, g * 128:(g + 1) * 128],
                                     rhs=dtraw[:, 0:Tn], start=True, stop=True)
                    esp = scr.tile([128, L], FP, tag="esp", name="esp")
                    nc.scalar.activation(out=esp[:, 0:Tn], in_=ps[:, 0:Tn],
                                         func=AF.Exp, bias=bdt_t[g])
                    nc.scalar.activation(out=dtT[g][:, 0:Tn], in_=esp[:, 0:Tn],
                                         func=AF.Ln, bias=1.0)
                    nc.vector.tensor_tensor(out=duT[g][:, 0:Tn],
                                            in0=dtT[g][:, 0:Tn],
                                            in1=xcT[g][:, 0:Tn], op=OP.mult)

                dap = dbl_dram[:, :]
                el = dap.ap[-1][0]

                yT = [sing.tile([128, L], FP, tag=f"yT{g}", name=f"yT{g}") for g in range(NB)]
                small = last and not rev
                yT = None
                yTl = [sing.tile([128, L], FP, tag=f"yT{g}", name=f"yT{g}")
                       for g in range(NB)]
                yt2 = scr.tile([128, L], FP, tag="yt2", name="yt2")
                for nh in range(4):
                    # broadcast B/C halves for this mamba
                    def bcast(dst, row0):
                        src = bass.AP(tensor=dap.tensor,
                                      offset=dap.offset + row0 * L * el,
                                      ap=[[0, 128], [L * el, NH2], [el, Tn]])
                        nc.sync.dma_start(out=dst[:, :, 0:Tn], in_=src)
                    bcast(B_rep, DTR + nh * NH2)
                    if not last:
                        bcast(C_rep, DTR + DS + nh * NH2)
                    for g in range(NB):
                        if small:
                            A2s = scr.tile([128, NH2, 2], BF, tag="A2s", name="A2s")
                            dBu2s = scr.tile([128, NH2, 2], BF, tag="dBu2s",
                                             name="dBu2s")
                            At, dBt, Ht2 = A2s, dBu2s, dBu2s
                            AL = 2
                        else:
                            At, dBt, Ht2 = A_blk, dBu_blk, dBu_blk
                            AL = L
                        for n in range(NH2):
                            nc.scalar.activation(out=At[:, n, 0:Tn],
                                                 in_=dtT[g][:, 0:Tn], func=AF.Exp,
                                                 scale=-float(nh * NH2 + n + 1))
                        ael = At.ap[-1][0]
                        t0 = 0 if not rev else Tn - 1
                        mask = bass.AP(tensor=At.tensor,
                                       offset=At.offset + t0 * ael,
                                       ap=[At.ap[0], [AL * ael, NH2], [ael, 1]])
                        nc.vector.memset(mask, 0.0)
                        del_ = duT[g].ap[-1][0]
                        du_s0 = bass.AP(tensor=duT[g].tensor, offset=duT[g].offset,
                                        ap=[duT[g].ap[0], [0, NH2], [del_, Tn]])
                        nc.vector.tensor_tensor(out=dBt[:, :, 0:Tn], in0=du_s0,
                                                in1=B_rep[:, :, 0:Tn], op=OP.mult)
                        if not small:
                            if not rev:
                                nc.vector.tensor_tensor_scan(
                                    out=flat2(dBu_blk, NH2 * L),
                                    data0=flat2(A_blk, NH2 * L),
                                    data1=flat2(dBu_blk, NH2 * L), initial=0.0,
                                    op0=OP.mult, op1=OP.add)
                            else:
                                nc.vector.tensor_tensor_scan(
                                    out=rev3(dBu_blk), data0=rev3(A_blk),
                                    data1=rev3(dBu_blk), initial=0.0,
                                    op0=OP.mult, op1=OP.add)
                        else:
                            nc.vector.tensor_tensor_scan(
                                out=flat2(dBu2s, NH2 * 2), data0=flat2(A2s, NH2 * 2),
                                data1=flat2(dBu2s, NH2 * 2), initial=0.0,
                                op0=OP.mult, op1=OP.add)
                        ytarget = yTl[g] if nh == 0 else yt2
                        if not last:
                            ych = A_blk  # reuse A slot (dead after scan)
                            nc.vector.tensor_tensor(out=ych, in0=Ht2, in1=C_rep,
                                                    op=OP.mult)
                            yel = ych.ap[-1][0]
                            red_in = bass.AP(tensor=ych.tensor, offset=ych.offset,
                                             ap=[ych.ap[0], [yel, L],
                                                 [L * yel, NH2]])
                            nc.vector.tensor_reduce(out=ytarget, in_=red_in,
                                                    axis=mybir.AxisListType.X,
                                                    op=OP.add)
                        else:
                            if small:
                                h_sl = Ht2[:, :, :]
                            else:
                                hel = Ht2.ap[-1][0]
                                h_sl = bass.AP(tensor=Ht2.tensor, offset=Ht2.offset,
                                               ap=[Ht2.ap[0], [L * hel, NH2],
                                                   [hel, 2]])
                            c2t = scr.tile([128, NH2, 2], BF, tag="c2t", name="c2t")
                            csrc = bass.AP(
                                tensor=dap.tensor,
                                offset=dap.offset + (DTR + DS + nh * NH2) * L * el,
                                ap=[[0, 128], [L * el, NH2], [el, 2]])
                            nc.sync.dma_start(out=c2t, in_=csrc)
                            tmp = scr.tile([128, NH2, 2], BF, tag="ychs",
                                           name="ychs")
                            nc.vector.tensor_tensor(out=tmp, in0=h_sl, in1=c2t,
                                                    op=OP.mult)
                            tel = tmp.ap[-1][0]
                            red_in = bass.AP(tensor=tmp.tensor, offset=tmp.offset,
                                             ap=[tmp.ap[0], [tel, 2],
                                                 [2 * tel, NH2]])
                            nc.vector.tensor_reduce(out=ytarget[:, 0:2],
                                                    in_=red_in,
                                                    axis=mybir.AxisListType.X,
                                                    op=OP.add)
                        if nh > 0:
                            Ty = 2 if last else L
                            nc.vector.tensor_tensor(out=yTl[g][:, 0:Ty],
                                                    in0=yTl[g][:, 0:Ty],
                                                    in1=yt2[:, 0:Ty], op=OP.add)
                yT = yTl

                # gate: g = (y + xc) * zsil  -> bf16
                gT = [scr.tile([128, L], BF, tag=f"gT{g}", name=f"gT{g}") for g in range(NB)]
                Tg = 2 if last else L
                for g in range(NB):
                    nc.vector.tensor_tensor(out=yT[g][:, 0:Tg], in0=yT[g][:, 0:Tg],
                                            in1=xcT[g][:, 0:Tg], op=OP.add)
                    nc.vector.tensor_tensor(out=gT[g][:, 0:Tg], in0=yT[g][:, 0:Tg],
                                            in1=zsil[g][:, 0:Tg], op=OP.mult)
                return gT, Tg

            def emit_layer(li):
                last = li == 1
                h_bf = [scr1.tile([128, L], BF, tag=f"hbf{g}", name=f"hbf{g}") for g in range(NB)]
                for g in range(NB):
                    nc.vector.tensor_copy(out=h_bf[g], in_=hT[g])
                g_f, Tg_f = emit_mamba(li, 0, h_bf, last)
                g_r, Tg_r = emit_mamba(li, 1, h_bf, last)
                Tm = 2 if last else L
                pso = [psacc.tile([128, L], FP, tag="acc", name="acc")
                       for _ in range(NB)]
                for dd, gg in ((0, g_f), (1, g_r)):
                    Wd = wload(f"Wout{li}{dd}", DM, DM, tag="wout")
                    for m in range(NB):
                        for k in range(NB):
                            nc.tensor.matmul(
                                pso[m][:, 0:Tm],
                                lhsT=Wd[k][:, m * 128:(m + 1) * 128],
                                rhs=gg[k][:, 0:Tm], start=(dd == 0 and k == 0),
                                stop=(dd == 1 and k == NB - 1))
                for m in range(NB):
                    nc.vector.tensor_tensor(out=hT[m][:, 0:Tm],
                                            in0=hT[m][:, 0:Tm], in1=pso[m][:, 0:Tm],
                                            op=OP.add)
                ln_inplace(Tm)
                ffn(li, Tm, last)

            def ln_inplace(T):
                """layernorm over d (partitions) of hT[:, 0:T], in place."""
                psm = pss.tile([1, L], FP, tag="sm", name="sm")
                psq = pss.tile([1, L], FP, tag="sm", name="sm")
                for g in range(NB):
                    sq = scr.tile([128, L], FP, tag="lnsq", name="lnsq")
                    nc.scalar.activation(out=sq[:, 0:T], in_=hT[g][:, 0:T],
                                         func=AF.Square)
                    nc.tensor.matmul(psm[:, 0:T], lhsT=ones_c, rhs=hT[g][:, 0:T],
                                     start=(g == 0), stop=(g == NB - 1))
                    nc.tensor.matmul(psq[:, 0:T], lhsT=ones_c, rhs=sq[:, 0:T],
                                     start=(g == 0), stop=(g == NB - 1))
                mean = scr.tile([1, L], FP, tag="lnmean", name="lnmean")
                nc.vector.tensor_scalar(out=mean[:, 0:T], in0=psm[:, 0:T],
                                        scalar1=1.0 / DM, scalar2=None, op0=OP.mult)
                m2 = scr.tile([1, L], FP, tag="lnm2", name="lnm2")
                nc.vector.tensor_tensor(out=m2[:, 0:T], in0=mean[:, 0:T],
                                        in1=mean[:, 0:T], op=OP.mult)
                var = scr.tile([1, L], FP, tag="lnvar", name="lnvar")
                nc.vector.scalar_tensor_tensor(out=var[:, 0:T], in0=psq[:, 0:T],
                                               scalar=1.0 / DM, in1=m2[:, 0:T],
                                               op0=OP.mult, op1=OP.subtract)
                sd = scr.tile([1, L], FP, tag="lnsd", name="lnsd")
                nc.scalar.activation(out=sd[:, 0:T], in_=var[:, 0:T],
                                     func=AF.Sqrt, bias=eps_t)
                rinv = scr.tile([1, L], FP, tag="lnrinv", name="lnrinv")
                nc.vector.reciprocal(out=rinv[:, 0:T], in_=sd[:, 0:T])
                mrep = psum.tile([128, L], FP, tag="tr", name="tr")
                nc.tensor.matmul(mrep[:, 0:T], lhsT=ones_r, rhs=mean[:, 0:T],
                                 start=True, stop=True)
                rrep = psum.tile([128, L], FP, tag="tr", name="tr")
                nc.tensor.matmul(rrep[:, 0:T], lhsT=ones_r, rhs=rinv[:, 0:T],
                                 start=True, stop=True)
                mrs = scr.tile([128, L], FP, tag="lnmrs", name="lnmrs")
                nc.scalar.copy(out=mrs[:, 0:T], in_=mrep[:, 0:T])
                rrs = scr.tile([128, L], FP, tag="lnrrs", name="lnrrs")
                nc.scalar.copy(out=rrs[:, 0:T], in_=rrep[:, 0:T])
                for g in range(NB):
                    c = scr.tile([128, L], FP, tag="lnc", name="lnc")
                    nc.vector.tensor_tensor(out=c[:, 0:T], in0=hT[g][:, 0:T],
                                            in1=mrs[:, 0:T], op=OP.subtract)
                    nc.vector.tensor_tensor(out=hT[g][:, 0:T], in0=c[:, 0:T],
                                            in1=rrs[:, 0:T], op=OP.mult)

            def ffn(li, T, last):
                h_bf = [scr1.tile([128, L], BF, tag=f"fhbf{g}", name=f"fhbf{g}") for g in range(NB)]
                for g in range(NB):
                    nc.vector.tensor_copy(out=h_bf[g][:, 0:T], in_=hT[g][:, 0:T])
                b1 = vec(f"ffb1_{li}", DF)
                b2 = vec(f"ffb2_{li}")
                pso = [psacc.tile([128, L], FP, tag="acc", name="acc")
                       for _ in range(NB)]
                for half in range(4):
                    W1 = []
                    for k in range(NB):
                        t = wpool.tile([128, DF // 4], BF, tag=f"ffw1_{k}",
                                       name=f"ffw1_{k}")
                        nc.sync.dma_start(
                            out=t, in_=P[f"ffW1_{li}"][k * 128:(k + 1) * 128,
                                                       half * (DF // 4):
                                                       (half + 1) * (DF // 4)])
                        W1.append(t)
                    yb = [scr1.tile([128, L], BF, tag=f"ffyb{k}", name=f"ffyb{k}")
                          for k in range(4)]
                    for k8 in range(4):
                        m = half * 4 + k8
                        ps = psum.tile([128, L], FP, tag="tr", name="tr")
                        for k in range(NB):
                            nc.tensor.matmul(ps[:, 0:T],
                                             lhsT=W1[k][:, k8 * 128:(k8 + 1) * 128],
                                             rhs=h_bf[k][:, 0:T], start=(k == 0),
                                             stop=(k == NB - 1))
                        nc.scalar.activation(out=yb[k8][:, 0:T], in_=ps[:, 0:T],
                                             func=AF.Relu, bias=b1[m])
                    W2h = []
                    for k8 in range(4):
                        t = wpool.tile([128, DM], BF, tag=f"ffw2_{k8}",
                                       name=f"ffw2_{k8}")
                        r0 = (half * 4 + k8) * 128
                        nc.sync.dma_start(out=t,
                                          in_=P[f"ffW2_{li}"][r0:r0 + 128, :])
                        W2h.append(t)
                    for m in range(NB):
                        for k8 in range(4):
                            nc.tensor.matmul(
                                pso[m][:, 0:T],
                                lhsT=W2h[k8][:, m * 128:(m + 1) * 128],
                                rhs=yb[k8][:, 0:T], start=(half == 0 and k8 == 0),
                                stop=(half == 3 and k8 == 3))
                for m in range(NB):
                    nc.vector.scalar_tensor_tensor(out=hT[m][:, 0:T],
                                                   in0=pso[m][:, 0:T], scalar=b2[m],
                                                   in1=hT[m][:, 0:T], op0=OP.add,
                                                   op1=OP.add)
                ln_inplace(T)

            emit_layer(0)
            emit_layer(1)

            # final projection at positions 0,1
            h_bf = [scr.tile([128, 2], BF, tag=f"pjb{g}", name=f"pjb{g}") for g in range(NB)]
            for g in range(NB):
                nc.vector.tensor_copy(out=h_bf[g], in_=hT[g][:, 0:2])
            PW = wload("projW", DM, PRED, tag="w512")
            pb = sing.tile([PRED, 1], FP)
            nc.sync.dma_start(out=pb, in_=P["projb"][:])
            ps = pss.tile([PRED, 2], FP, tag="sm", name="sm")
            for k in range(NB):
                nc.tensor.matmul(ps, lhsT=PW[k], rhs=h_bf[k], start=(k == 0),
                                 stop=(k == NB - 1))
            res = sing.tile([PRED, 2], FP)
            nc.vector.tensor_scalar(out=res, in0=ps, scalar1=pb, scalar2=None,
                                    op0=OP.add)
            nc.sync.dma_start(out=out_d[:, :], in_=res)

    nc.finalize()
    return nc


_CACHE = {}


def kernel(**inputs):
    w, xts, means, stdev = prep_host_inputs(inputs)
    if "nc" not in _CACHE:
        _CACHE["nc"] = build_program()
    nc = _CACHE["nc"]
    in_maps = []
    for b in range(8):
        m = dict(w)
        m["xT"] = xts[b]
        in_maps.append(m)
    rr = run_bass_kernel_spmd(nc, in_maps, list(range(8)))
    outs = []
    for b in range(8):
        o = np.asarray(rr.results[b]["out"], np.float32)     # [96, 2]
        o = o * stdev[b][None, :] + means[b][None, :]
        outs.append(o)
    return np.stack(outs)                                    # [8, 96, 2]


# revision 19
# speedup vs baseline: 1.1165x; 1.1165x over previous
"""Trainium2 Bass kernel for nn_Experiment6 (bi-mamba + MHA + FFN forecaster).

Sharding: data-parallel over batch (B=8) across 8 NeuronCores; all params
replicated. Inside each core: activations kept transposed [feature, time];
selective scan via DVE tensor_tensor_scan in n-major layout
[128 d-partitions, (n=16, t=512) free]; reverse-direction mamba handled with
reversed free-axis APs (no data reversal). Output depends only on positions
0,1 of the final sequence, so the last layer is pruned accordingly.
RevIN normalization and final rescale are host-side (exact fp32).
"""
import numpy as np

import concourse.bacc as bacc
import concourse.bass as bass
import concourse.tile as tile
from concourse import mybir
from concourse.bass_utils import run_bass_kernel_spmd

FP = mybir.dt.float32
BF = mybir.dt.bfloat16
AF = mybir.ActivationFunctionType
OP = mybir.AluOpType

L = 512
DM = 512
DS = 16
DF = 2048
DTR = 32
NH = 4
DH = 128
PRED = 96
EPS = 1e-5
NB = 4  # number of 128-partition blocks in DM


def _f(x):
    return np.ascontiguousarray(np.asarray(x, np.float32))


def _bf(x):
    import ml_dtypes
    return np.ascontiguousarray(np.asarray(x, np.float32).astype(ml_dtypes.bfloat16))


def prep_host_inputs(inputs):
    """Returns (shared weight map, per-core x maps, per-core (mean, std))."""
    w = {}
    w["Wp"] = _bf(inputs["Wp"])                                # [2, 512]
    w["bp"] = _f(inputs["bp"])
    s = 1.0 / np.sqrt(DH)
    w["Wq"] = _bf(_f(inputs["Wq"]) * s)
    w["bq"] = _f(_f(inputs["bq"]) * s)
    w["Wk"] = _bf(inputs["Wk"])
    w["bk"] = _f(inputs["bk"])
    w["Wv"] = _bf(inputs["Wv"])
    w["Wo"] = _bf(inputs["Wo"])
    # fold v-bias through Wo, plus bi (the empty-input branch bias)
    bo2 = _f(inputs["bo"]) + _f(inputs["bi"]) + _f(inputs["Wo"]).T @ _f(inputs["bv"])
    w["bo2"] = _f(bo2)
    for li in range(2):
        for dd in range(2):
            tag = f"{li}{dd}"
            w["Win" + tag] = _bf(inputs["m_Win"][li, dd])       # [512, 1024]
            w["convw" + tag] = _f(inputs["m_convw"][li, dd])    # [512, 2]
            w["convb" + tag] = _f(inputs["m_convb"][li, dd])    # [512]
            w["Wx" + tag] = _bf(inputs["m_Wx"][li, dd])         # [512, 64]
            w["Wdt" + tag] = _bf(inputs["m_Wdt"][li, dd])       # [32, 512]
            w["bdt" + tag] = _f(inputs["m_bdt"][li, dd])        # [512]
            w["Wout" + tag] = _bf(inputs["m_Wout"][li, dd])     # [512, 512]
    for li in range(2):
        w[f"ffW1_{li}"] = _bf(inputs["ff_W1"][li])              # [512, 2048]
        w[f"ffb1_{li}"] = _f(inputs["ff_b1"][li])
        w[f"ffW2_{li}"] = _bf(inputs["ff_W2"][li])              # [2048, 512]
        w[f"ffb2_{li}"] = _f(inputs["ff_b2"][li])
    w["projW"] = _bf(inputs["proj_W"])                          # [512, 96]
    w["projb"] = _f(inputs["proj_b"])

    x_enc = _f(inputs["x_enc"])                                 # [8, 512, 2]
    means = x_enc.mean(1, keepdims=True)                        # [8,1,2]
    xc = x_enc - means
    stdev = np.sqrt(xc.var(axis=1, keepdims=True) + 1e-5)
    xn = xc / stdev
    xts = [np.ascontiguousarray(xn[b].T) for b in range(8)]     # [2,512] each
    return w, xts, means[:, 0, :], stdev[:, 0, :]


def rev3(t):
    """Flat reversed AP over a contiguous [128, 16, 512] n-major tile: iterates
    (n desc, t desc) so each n-chain runs t-descending; block transitions are
    cut by the a=0 mask at t=511."""
    el = t.ap[-1][0]
    ntot = t.shape[1] * t.shape[2]
    return bass.AP(tensor=t.tensor, offset=t.offset + (ntot - 1) * el,
                   ap=[t.ap[0], [-el, ntot]])


def flat2(t, ntot):
    el = t.ap[-1][0]
    return bass.AP(tensor=t.tensor, offset=t.offset, ap=[t.ap[0], [el, ntot]])


def build_program():
    nc = bacc.Bacc()
    P = {}

    def par(name, shape, dt):
        P[name] = nc.declare_dram_parameter(name, list(shape), dt, isOutput=False)
        return P[name]

    par("xT", (2, L), FP)
    par("Wp", (2, DM), BF); par("bp", (DM,), FP)
    for nm in ("Wq", "Wk", "Wv", "Wo"):
        par(nm, (DM, DM), BF)
    par("bq", (DM,), FP); par("bk", (DM,), FP); par("bo2", (DM,), FP)
    for li in range(2):
        for dd in range(2):
            tg = f"{li}{dd}"
            par("Win" + tg, (DM, 2 * DM), BF)
            par("convw" + tg, (DM, 2), FP)
            par("convb" + tg, (DM,), FP)
            par("Wx" + tg, (DM, DTR + 2 * DS), BF)
            par("Wdt" + tg, (DTR, DM), BF)
            par("bdt" + tg, (DM,), FP)
            par("Wout" + tg, (DM, DM), BF)
    for li in range(2):
        par(f"ffW1_{li}", (DM, DF), BF); par(f"ffb1_{li}", (DF,), FP)
        par(f"ffW2_{li}", (DF, DM), BF); par(f"ffb2_{li}", (DM,), FP)
    par("projW", (DM, PRED), BF); par("projb", (PRED,), FP)
    out_d = nc.declare_dram_parameter("out", [PRED, 2], FP, isOutput=True)

    with tile.TileContext(nc) as tc:
        import contextlib
        ctx = contextlib.ExitStack()
        with ctx:
            sing = ctx.enter_context(tc.tile_pool(name="sing", bufs=1))
            scr = ctx.enter_context(tc.tile_pool(name="scr", bufs=2))
            scr1 = ctx.enter_context(tc.tile_pool(name="scr1", bufs=1))
            bigp = ctx.enter_context(tc.tile_pool(name="bigp", bufs=2))
            wpool = ctx.enter_context(tc.tile_pool(name="wp", bufs=1))
            big = ctx.enter_context(tc.tile_pool(name="big", bufs=1))
            psum = ctx.enter_context(tc.tile_pool(name="ps", bufs=2, space="PSUM"))
            psacc = ctx.enter_context(tc.tile_pool(name="psacc", bufs=4, space="PSUM"))
            pss = ctx.enter_context(tc.tile_pool(name="pss", bufs=2, space="PSUM"))
            dram = ctx.enter_context(tc.tile_pool(name="dr", bufs=1, space="DRAM"))

            def vec(name, n=DM, dt=FP):
                """load a DRAM vector as NB [128,1] bias tiles"""
                ts = []
                for g in range(n // 128):
                    t = sing.tile([128, 1], dt, tag=f"v_{name}_{g}", name=f"v_{name}_{g}")
                    nc.sync.dma_start(out=t, in_=P[name][g * 128:(g + 1) * 128])
                    ts.append(t)
                return ts

            def wload(name, rows, cols, tag=None, dt=BF):
                """load weight [rows, cols] as rows//128 k-tiles"""
                ts = []
                nk = max(1, rows // 128)
                kr = rows // nk
                for k in range(nk):
                    t = wpool.tile([kr, cols], dt, tag=(tag or name) + f"_{k}")
                    nc.sync.dma_start(out=t, in_=P[name][k * kr:(k + 1) * kr, :])
                    ts.append(t)
                return ts

            ones_c = sing.tile([128, 1], FP)
            nc.vector.memset(ones_c, 1.0)
            ones_r = sing.tile([1, 128], FP)
            nc.vector.memset(ones_r, 1.0)
            eps_t = sing.tile([1, 1], FP)
            nc.vector.memset(eps_t, EPS)

            # ---- embed: ppT = Wp^T @ xT + bp ----
            xT = sing.tile([2, L], FP)
            nc.sync.dma_start(out=xT, in_=P["xT"][:, :])
            xTb = sing.tile([2, L], BF)
            nc.vector.tensor_copy(out=xTb, in_=xT)
            Wp_t = wload("Wp", 2, DM, tag="wp512x")  # [2, 512] single tile (rows<128)
            bp_t = vec("bp")
            pp_bf = [sing.tile([128, L], BF, tag=f"ppbf{g}", name=f"ppbf{g}") for g in range(NB)]
            for g in range(NB):
                ps = psum.tile([128, L], FP, tag="tr", name="tr")
                nc.tensor.matmul(ps, lhsT=Wp_t[0][:, g * 128:(g + 1) * 128],
                                 rhs=xTb, start=True, stop=True)
                nc.vector.tensor_scalar(out=pp_bf[g], in0=ps, scalar1=bp_t[g],
                                        scalar2=None, op0=OP.add)

            # ---- MHA ----
            def proj_T(wname, bias_ts, outdt=BF):
                """outT[do, t] = W^T @ pp (+bias): returns NB tiles"""
                Wt = wload(wname, DM, DM, tag="w512")
                outs = []
                for m in range(NB):
                    ps = psum.tile([128, L], FP, tag="tr", name="tr")
                    for k in range(NB):
                        nc.tensor.matmul(ps, lhsT=Wt[k][:, m * 128:(m + 1) * 128],
                                         rhs=pp_bf[k], start=(k == 0),
                                         stop=(k == NB - 1))
                    o = sing.tile([128, L], outdt, tag=f"{wname}_o{m}", name=f"{wname}_o{m}")
                    if bias_ts is None:
                        nc.scalar.copy(out=o, in_=ps)
                    else:
                        nc.vector.tensor_scalar(out=o, in0=ps, scalar1=bias_ts[m],
                                                scalar2=None, op0=OP.add)
                    outs.append(o)
                return outs

            qT = proj_T("Wq", vec("bq"))
            kT = proj_T("Wk", vec("bk"))
            # V in natural layout: V[t, d] = pp[t, :] @ Wv
            Wv_t = wload("Wv", DM, DM, tag="w512")
            Vn = []
            for m in range(NB):  # m indexes t-blocks
                ps = psum.tile([128, L], FP, tag="tr", name="tr")
                for k in range(NB):
                    nc.tensor.matmul(ps, lhsT=pp_bf[k][:, m * 128:(m + 1) * 128],
                                     rhs=Wv_t[k], start=(k == 0), stop=(k == NB - 1))
                o = sing.tile([128, L], BF, tag=f"vn{m}", name=f"vn{m}")
                nc.scalar.copy(out=o, in_=ps)
                Vn.append(o)

            oT = [sing.tile([128, L], BF, tag=f"oT{h}", name=f"oT{h}") for h in range(NH)]
            for h in range(NH):
                # ST[m, l] = K_h^T Q_h ; E = exp(ST); denom = ones^T E
                E_h = []
                dn = pss.tile([1, L], FP, tag="sm", name="sm")
                for mb in range(NB):
                    ps = psum.tile([128, L], FP, tag="tr", name="tr")
                    nc.tensor.matmul(ps, lhsT=kT[h][:, mb * 128:(mb + 1) * 128],
                                     rhs=qT[h], start=True, stop=True)
                    e = scr1.tile([128, L], BF, tag=f"eh{mb}", name=f"eh{mb}")
                    nc.scalar.activation(out=e, in_=ps, func=AF.Exp)
                    E_h.append(e)
                ob = scr.tile([1, 128], BF, tag="onesbf", name="onesbf")
                nc.vector.tensor_copy(out=ob, in_=ones_r)
                oc = scr.tile([128, 1], BF, tag="onescbf", name="onescbf")
                nc.vector.tensor_copy(out=oc, in_=ones_c)
                for mb in range(NB):
                    nc.tensor.matmul(dn, lhsT=oc, rhs=E_h[mb],
                                     start=(mb == 0), stop=(mb == NB - 1))
                rinv = scr.tile([1, L], FP, tag="rinv", name="rinv")
                nc.vector.reciprocal(out=rinv, in_=dn)
                rb = scr.tile([1, L], BF, tag="rb", name="rb")
                nc.vector.tensor_copy(out=rb, in_=rinv)
                rrep = psum.tile([128, L], FP, tag="tr", name="tr")
                nc.tensor.matmul(rrep, lhsT=ob, rhs=rb, start=True, stop=True)
                rrs = scr.tile([128, L], FP, tag="rrs", name="rrs")
                nc.scalar.copy(out=rrs, in_=rrep)
                # AV: OT_h = sum_m V[m, dh] E[m, l]
                av = psum.tile([128, L], FP, tag="tr", name="tr")
                for mb in range(NB):
                    nc.tensor.matmul(av, lhsT=Vn[mb][:, h * 128:(h + 1) * 128],
                                     rhs=E_h[mb], start=(mb == 0),
                                     stop=(mb == NB - 1))
                nc.vector.tensor_tensor(out=oT[h], in0=av, in1=rrs, op=OP.mult)

            bo2_t = vec("bo2")
            Wo_t = wload("Wo", DM, DM, tag="w512")
            hT = [sing.tile([128, L], FP, tag=f"hT{g}", name=f"hT{g}") for g in range(NB)]
            for m in range(NB):
                ps = psum.tile([128, L], FP, tag="tr", name="tr")
                for k in range(NB):
                    nc.tensor.matmul(ps, lhsT=Wo_t[k][:, m * 128:(m + 1) * 128],
                                     rhs=oT[k], start=(k == 0), stop=(k == NB - 1))
                nc.vector.tensor_scalar(out=hT[m], in0=ps, scalar1=bo2_t[m],
                                        scalar2=None, op0=OP.add)

            # ---- persistent mamba tiles ----
            NH2 = DS // 4
            B_rep = big.tile([128, NH2, L], BF, tag="Brep", name="Brep")
            C_rep = big.tile([128, NH2, L], BF, tag="Crep", name="Crep")
            dbl_dram = dram.tile([64, L], BF, tag="dbldram", name="dbldram")

            def emit_mamba(li, dd, h_bf, last):
                tg = f"{li}{dd}"
                rev = dd == 1
                Tn = 2 if (last and not rev) else L
                # Win matmuls: x-half always full T (rev) or Tn; z-half Tn2
                def win_half(co):
                    ts = []
                    for k in range(NB):
                        t = wpool.tile([128, DM], BF, tag=f"win_{k}",
                                       name=f"win_{k}")
                        nc.sync.dma_start(
                            out=t, in_=P["Win" + tg][k * 128:(k + 1) * 128,
                                                     co:co + DM])
                        ts.append(t)
                    return ts

                Win_t = win_half(0)
                Tx = L if not last or rev else 3
                xcpre = []
                for m in range(NB):
                    ps = psacc.tile([128, L], FP, tag="acc", name="acc")
                    for k in range(NB):
                        nc.tensor.matmul(ps[:, 0:Tx],
                                         lhsT=Win_t[k][:, m * 128:(m + 1) * 128],
                                         rhs=h_bf[k][:, 0:Tx], start=(k == 0),
                                         stop=(k == NB - 1))
                    xcpre.append(ps)
                Tz = 2 if last else L
                Win_z = win_half(DM)
                zsil = []
                for m in range(NB):
                    ps = psum.tile([128, L], FP, tag="tr", name="tr")
                    for k in range(NB):
                        nc.tensor.matmul(
                            ps[:, 0:Tz],
                            lhsT=Win_z[k][:, m * 128:(m + 1) * 128],
                            rhs=h_bf[k][:, 0:Tz], start=(k == 0), stop=(k == NB - 1))
                    o = sing.tile([128, L], BF, tag=f"zsil{m}", name=f"zsil{m}")
                    nc.scalar.activation(out=o[:, 0:Tz], in_=ps[:, 0:Tz], func=AF.Silu)
                    zsil.append(o)

                convw = P["convw" + tg]
                w0 = [sing.tile([128, 1], FP, tag=f"w0_{g}", name=f"w0_{g}") for g in range(NB)]
                w1 = [sing.tile([128, 1], FP, tag=f"w1_{g}", name=f"w1_{g}") for g in range(NB)]
                for g in range(NB):
                    nc.sync.dma_start(out=w0[g],
                                      in_=convw[g * 128:(g + 1) * 128, 0:1])
                    nc.sync.dma_start(out=w1[g],
                                      in_=convw[g * 128:(g + 1) * 128, 1:2])
                cb_t = vec("convb" + tg)
                xcT = [sing.tile([128, L], BF, tag=f"xcT{g}", name=f"xcT{g}") for g in range(NB)]
                Tc = Tx if (last and not rev) else L
                for g in range(NB):
                    t1 = scr.tile([128, L], FP, tag="convt1", name="convt1")
                    nc.vector.tensor_scalar(out=t1[:, 0:Tc], in0=xcpre[g][:, 0:Tc],
                                            scalar1=w1[g], scalar2=cb_t[g],
                                            op0=OP.mult, op1=OP.add)
                    c2 = scr.tile([128, L], FP, tag="convt2", name="convt2")
                    if not rev:
                        nc.vector.scalar_tensor_tensor(
                            out=c2[:, 1:Tc], in0=xcpre[g][:, 0:Tc - 1],
                            scalar=w0[g], in1=t1[:, 1:Tc], op0=OP.mult, op1=OP.add)
                        nc.vector.tensor_copy(out=c2[:, 0:1], in_=t1[:, 0:1])
                    else:
                        nc.vector.scalar_tensor_tensor(
                            out=c2[:, 0:Tc - 1], in0=xcpre[g][:, 1:Tc],
                            scalar=w0[g], in1=t1[:, 0:Tc - 1], op0=OP.mult,
                            op1=OP.add)
                        nc.vector.tensor_copy(out=c2[:, Tc - 1:Tc],
                                              in_=t1[:, Tc - 1:Tc])
                    nc.scalar.activation(out=xcT[g][:, 0:Tn], in_=c2[:, 0:Tn],
                                         func=AF.Silu)

                # dbl = Wx^T @ xc  [64, Tn]
                Wx_t = wload("Wx" + tg, DM, 64, tag="wx")
                psd = pss.tile([64, L], FP, tag="sm", name="sm")
                for k in range(NB):
                    nc.tensor.matmul(psd[:, 0:Tn], lhsT=Wx_t[k],
                                     rhs=xcT[k][:, 0:Tn],
                                     start=(k == 0), stop=(k == NB - 1))
                dblT = scr.tile([64, L], FP, tag="dblT", name="dblT")
                nc.scalar.copy(out=dblT[:, 0:Tn], in_=psd[:, 0:Tn])
                dbl_bf = scr.tile([64, L], BF, tag="dblbf", name="dblbf")
                nc.vector.tensor_copy(out=dbl_bf[:, 0:Tn], in_=dblT[:, 0:Tn])
                nc.sync.dma_start(out=dbl_dram[:, 0:Tn], in_=dbl_bf[:, 0:Tn])
                dtraw = scr.tile([DTR, L], BF, tag="dtraw", name="dtraw")
                nc.vector.tensor_copy(out=dtraw[:, 0:Tn], in_=dblT[0:DTR, 0:Tn])

                # dt = softplus(Wdt^T @ dtraw + bdt)
                Wdt_t = wload("Wdt" + tg, DTR, DM, tag="wdt512")
                bdt_t = vec("bdt" + tg)
                dtT = [sing.tile([128, L], FP, tag=f"dtT{g}", name=f"dtT{g}") for g in range(NB)]
                duT = [sing.tile([128, L], BF, tag=f"duT{g}", name=f"duT{g}") for g in range(NB)]
                for g in range(NB):
                    ps = psum.tile([128, L], FP, tag="tr", name="tr")
                    nc.tensor.matmul(ps[:, 0:Tn],
                                     lhsT=Wdt_t[0][:, g * 128:(g + 1) * 128],
                                     rhs=dtraw[:, 0:Tn], start=True, stop=True)
                    esp = scr.tile([128, L], FP, tag="esp", name="esp")
                    nc.scalar.activation(out=esp[:, 0:Tn], in_=ps[:, 0:Tn],
                                         func=AF.Exp, bias=bdt_t[g])
                    nc.scalar.activation(out=dtT[g][:, 0:Tn], in_=esp[:, 0:Tn],
                                         func=AF.Ln, bias=1.0)
                    nc.vector.tensor_tensor(out=duT[g][:, 0:Tn],
                                            in0=dtT[g][:, 0:Tn],
                                            in1=xcT[g][:, 0:Tn], op=OP.mult)

                dap = dbl_dram[:, :]
                el = dap.ap[-1][0]

                yT = [sing.tile([128, L], FP, tag=f"yT{g}", name=f"yT{g}") for g in range(NB)]
                small = last and not rev
                yT = None
                yTl = [sing.tile([128, L], FP, tag=f"yT{g}", name=f"yT{g}")
                       for g in range(NB)]
                yt2 = scr.tile([128, L], FP, tag="yt2", name="yt2")
                for nh in range(4):
                    # broadcast B/C halves for this mamba
                    def bcast(dst, row0):
                        src = bass.AP(tensor=dap.tensor,
                                      offset=dap.offset + row0 * L * el,
                                      ap=[[0, 128], [L * el, NH2], [el, Tn]])
                        nc.sync.dma_start(out=dst[:, :, 0:Tn], in_=src)
                    bcast(B_rep, DTR + nh * NH2)
                    if not last:
                        bcast(C_rep, DTR + DS + nh * NH2)
                    for g in range(NB):
                        if small:
                            A2s = scr.tile([128, NH2, 2], BF, tag="A2s", name="A2s")
                            dBu2s = scr.tile([128, NH2, 2], BF, tag="dBu2s",
                                             name="dBu2s")
                            At, dBt, Ht2 = A2s, dBu2s, dBu2s
                            AL = 2
                        else:
                            A_blk = bigp.tile([128, NH2, L], BF, tag="Ablk",
                                              name="Ablk")
                            dBu_blk = bigp.tile([128, NH2, L], BF, tag="dBublk",
                                                name="dBublk")
                            At, dBt, Ht2 = A_blk, dBu_blk, dBu_blk
                            AL = L
                        for n in range(NH2):
                            nc.scalar.activation(out=At[:, n, 0:Tn],
                                                 in_=dtT[g][:, 0:Tn], func=AF.Exp,
                                                 scale=-float(nh * NH2 + n + 1))
                        ael = At.ap[-1][0]
                        t0 = 0 if not rev else Tn - 1
                        mask = bass.AP(tensor=At.tensor,
                                       offset=At.offset + t0 * ael,
                                       ap=[At.ap[0], [AL * ael, NH2], [ael, 1]])
                        nc.vector.memset(mask, 0.0)
                        del_ = duT[g].ap[-1][0]
                        du_s0 = bass.AP(tensor=duT[g].tensor, offset=duT[g].offset,
                                        ap=[duT[g].ap[0], [0, NH2], [del_, Tn]])
                        nc.vector.tensor_tensor(out=dBt[:, :, 0:Tn], in0=du_s0,
                                                in1=B_rep[:, :, 0:Tn], op=OP.mult)
                        if not small:
                            if not rev:
                                nc.vector.tensor_tensor_scan(
                                    out=flat2(dBu_blk, NH2 * L),
                                    data0=flat2(A_blk, NH2 * L),
                                    data1=flat2(dBu_blk, NH2 * L), initial=0.0,
                                    op0=OP.mult, op1=OP.add)
                            else:
                                nc.vector.tensor_tensor_scan(
                                    out=rev3(dBu_blk), data0=rev3(A_blk),
                                    data1=rev3(dBu_blk), initial=0.0,
                                    op0=OP.mult, op1=OP.add)
                        else:
                            nc.vector.tensor_tensor_scan(
                                out=flat2(dBu2s, NH2 * 2), data0=flat2(A2s, NH2 * 2),
                                data1=flat2(dBu2s, NH2 * 2), initial=0.0,
                                op0=OP.mult, op1=OP.add)
                        ytarget = yTl[g] if nh == 0 else yt2
                        if not last:
                            ych = Ht2  # in-place: H *= C_rep
                            nc.vector.tensor_tensor(out=ych, in0=Ht2, in1=C_rep,
                                                    op=OP.mult)
                            yel = ych.ap[-1][0]
                            red_in = bass.AP(tensor=ych.tensor, offset=ych.offset,
                                             ap=[ych.ap[0], [yel, L],
                                                 [L * yel, NH2]])
                            nc.vector.tensor_reduce(out=ytarget, in_=red_in,
                                                    axis=mybir.AxisListType.X,
                                                    op=OP.add)
                        else:
                            if small:
                                h_sl = Ht2[:, :, :]
                            else:
                                hel = Ht2.ap[-1][0]
                                h_sl = bass.AP(tensor=Ht2.tensor, offset=Ht2.offset,
                                               ap=[Ht2.ap[0], [L * hel, NH2],
                                                   [hel, 2]])
                            c2t = scr.tile([128, NH2, 2], BF, tag="c2t", name="c2t")
                            csrc = bass.AP(
                                tensor=dap.tensor,
                                offset=dap.offset + (DTR + DS + nh * NH2) * L * el,
                                ap=[[0, 128], [L * el, NH2], [el, 2]])
                            nc.sync.dma_start(out=c2t, in_=csrc)
                            tmp = scr.tile([128, NH2, 2], BF, tag="ychs",
                                           name="ychs")
                            nc.vector.tensor_tensor(out=tmp, in0=h_sl, in1=c2t,
                                                    op=OP.mult)
                            tel = tmp.ap[-1][0]
                            red_in = bass.AP(tensor=tmp.tensor, offset=tmp.offset,
                                             ap=[tmp.ap[0], [tel, 2],
                                                 [2 * tel, NH2]])
                            nc.vector.tensor_reduce(out=ytarget[:, 0:2],
                                                    in_=red_in,
                                                    axis=mybir.AxisListType.X,
                                                    op=OP.add)
                        if nh > 0:
                            Ty = 2 if last else L
                            nc.vector.tensor_tensor(out=yTl[g][:, 0:Ty],
                                                    in0=yTl[g][:, 0:Ty],
                                                    in1=yt2[:, 0:Ty], op=OP.add)
                yT = yTl

                # gate: g = (y + xc) * zsil  -> bf16
                gT = [scr.tile([128, L], BF, tag=f"gT{g}", name=f"gT{g}") for g in range(NB)]
                Tg = 2 if last else L
                for g in range(NB):
                    nc.vector.tensor_tensor(out=yT[g][:, 0:Tg], in0=yT[g][:, 0:Tg],
                                            in1=xcT[g][:, 0:Tg], op=OP.add)
                    nc.vector.tensor_tensor(out=gT[g][:, 0:Tg], in0=yT[g][:, 0:Tg],
                                            in1=zsil[g][:, 0:Tg], op=OP.mult)
                return gT, Tg

            def emit_layer(li):
                last = li == 1
                h_bf = [scr1.tile([128, L], BF, tag=f"hbf{g}", name=f"hbf{g}") for g in range(NB)]
                for g in range(NB):
                    nc.vector.tensor_copy(out=h_bf[g], in_=hT[g])
                g_f, Tg_f = emit_mamba(li, 0, h_bf, last)
                g_r, Tg_r = emit_mamba(li, 1, h_bf, last)
                Tm = 2 if last else L
                pso = [psacc.tile([128, L], FP, tag="acc", name="acc")
                       for _ in range(NB)]
                for dd, gg in ((0, g_f), (1, g_r)):
                    Wd = wload(f"Wout{li}{dd}", DM, DM, tag="wout")
                    for m in range(NB):
                        for k in range(NB):
                            nc.tensor.matmul(
                                pso[m][:, 0:Tm],
                                lhsT=Wd[k][:, m * 128:(m + 1) * 128],
                                rhs=gg[k][:, 0:Tm], start=(dd == 0 and k == 0),
                                stop=(dd == 1 and k == NB - 1))
                for m in range(NB):
                    nc.vector.tensor_tensor(out=hT[m][:, 0:Tm],
                                            in0=hT[m][:, 0:Tm], in1=pso[m][:, 0:Tm],
                                            op=OP.add)
                ln_inplace(Tm)
                ffn(li, Tm, last)

            def ln_inplace(T):
                """layernorm over d (partitions) of hT[:, 0:T], in place."""
                psm = pss.tile([1, L], FP, tag="sm", name="sm")
                psq = pss.tile([1, L], FP, tag="sm", name="sm")
                for g in range(NB):
                    sq = scr.tile([128, L], FP, tag="lnsq", name="lnsq")
                    nc.scalar.activation(out=sq[:, 0:T], in_=hT[g][:, 0:T],
                                         func=AF.Square)
                    nc.tensor.matmul(psm[:, 0:T], lhsT=ones_c, rhs=hT[g][:, 0:T],
                                     start=(g == 0), stop=(g == NB - 1))
                    nc.tensor.matmul(psq[:, 0:T], lhsT=ones_c, rhs=sq[:, 0:T],
                                     start=(g == 0), stop=(g == NB - 1))
                mean = scr.tile([1, L], FP, tag="lnmean", name="lnmean")
                nc.vector.tensor_scalar(out=mean[:, 0:T], in0=psm[:, 0:T],
                                        scalar1=1.0 / DM, scalar2=None, op0=OP.mult)
                m2 = scr.tile([1, L], FP, tag="lnm2", name="lnm2")
                nc.vector.tensor_tensor(out=m2[:, 0:T], in0=mean[:, 0:T],
                                        in1=mean[:, 0:T], op=OP.mult)
                var = scr.tile([1, L], FP, tag="lnvar", name="lnvar")
                nc.vector.scalar_tensor_tensor(out=var[:, 0:T], in0=psq[:, 0:T],
                                               scalar=1.0 / DM, in1=m2[:, 0:T],
                                               op0=OP.mult, op1=OP.subtract)
                sd = scr.tile([1, L], FP, tag="lnsd", name="lnsd")
                nc.scalar.activation(out=sd[:, 0:T], in_=var[:, 0:T],
                                     func=AF.Sqrt, bias=eps_t)
                rinv = scr.tile([1, L], FP, tag="lnrinv", name="lnrinv")
                nc.vector.reciprocal(out=rinv[:, 0:T], in_=sd[:, 0:T])
                mrep = psum.tile([128, L], FP, tag="tr", name="tr")
                nc.tensor.matmul(mrep[:, 0:T], lhsT=ones_r, rhs=mean[:, 0:T],
                                 start=True, stop=True)
                rrep = psum.tile([128, L], FP, tag="tr", name="tr")
                nc.tensor.matmul(rrep[:, 0:T], lhsT=ones_r, rhs=rinv[:, 0:T],
                                 start=True, stop=True)
                mrs = scr.tile([128, L], FP, tag="lnmrs", name="lnmrs")
                nc.scalar.copy(out=mrs[:, 0:T], in_=mrep[:, 0:T])
                rrs = scr.tile([128, L], FP, tag="lnrrs", name="lnrrs")
                nc.scalar.copy(out=rrs[:, 0:T], in_=rrep[:, 0:T])
                for g in range(NB):
                    c = scr.tile([128, L], FP, tag="lnc", name="lnc")
                    nc.vector.tensor_tensor(out=c[:, 0:T], in0=hT[g][:, 0:T],
                                            in1=mrs[:, 0:T], op=OP.subtract)
                    nc.vector.tensor_tensor(out=hT[g][:, 0:T], in0=c[:, 0:T],
                                            in1=rrs[:, 0:T], op=OP.mult)

            def ffn(li, T, last):
                h_bf = [scr1.tile([128, L], BF, tag=f"fhbf{g}", name=f"fhbf{g}") for g in range(NB)]
                for g in range(NB):
                    nc.vector.tensor_copy(out=h_bf[g][:, 0:T], in_=hT[g][:, 0:T])
                b1 = vec(f"ffb1_{li}", DF)
                b2 = vec(f"ffb2_{li}")
                pso = [psacc.tile([128, L], FP, tag="acc", name="acc")
                       for _ in range(NB)]
                for half in range(4):
                    W1 = []
                    for k in range(NB):
                        t = wpool.tile([128, DF // 4], BF, tag=f"ffw1_{k}",
                                       name=f"ffw1_{k}")
                        nc.sync.dma_start(
                            out=t, in_=P[f"ffW1_{li}"][k * 128:(k + 1) * 128,
                                                       half * (DF // 4):
                                                       (half + 1) * (DF // 4)])
                        W1.append(t)
                    yb = [scr1.tile([128, L], BF, tag=f"ffyb{k}", name=f"ffyb{k}")
                          for k in range(4)]
                    for k8 in range(4):
                        m = half * 4 + k8
                        ps = psum.tile([128, L], FP, tag="tr", name="tr")
                        for k in range(NB):
                            nc.tensor.matmul(ps[:, 0:T],
                                             lhsT=W1[k][:, k8 * 128:(k8 + 1) * 128],
                                             rhs=h_bf[k][:, 0:T], start=(k == 0),
                                             stop=(k == NB - 1))
                        nc.scalar.activation(out=yb[k8][:, 0:T], in_=ps[:, 0:T],
                                             func=AF.Relu, bias=b1[m])
                    W2h = []
                    for k8 in range(4):
                        t = wpool.tile([128, DM], BF, tag=f"ffw2_{k8}",
                                       name=f"ffw2_{k8}")
                        r0 = (half * 4 + k8) * 128
                        nc.sync.dma_start(out=t,
                                          in_=P[f"ffW2_{li}"][r0:r0 + 128, :])
                        W2h.append(t)
                    for m in range(NB):
                        for k8 in range(4):
                            nc.tensor.matmul(
                                pso[m][:, 0:T],
                                lhsT=W2h[k8][:, m * 128:(m + 1) * 128],
                                rhs=yb[k8][:, 0:T], start=(half == 0 and k8 == 0),
                                stop=(half == 3 and k8 == 3))
                for m in range(NB):
                    nc.vector.scalar_tensor_tensor(out=hT[m][:, 0:T],
                                                   in0=pso[m][:, 0:T], scalar=b2[m],
                                                   in1=hT[m][:, 0:T], op0=OP.add,
                                                   op1=OP.add)
                ln_inplace(T)

            emit_layer(0)
            emit_layer(1)

            # final projection at positions 0,1
            h_bf = [scr.tile([128, 2], BF, tag=f"pjb{g}", name=f"pjb{g}") for g in range(NB)]
            for g in range(NB):
                nc.vector.tensor_copy(out=h_bf[g], in_=hT[g][:, 0:2])
            PW = wload("projW", DM, PRED, tag="w512")
            pb = sing.tile([PRED, 1], FP)
            nc.sync.dma_start(out=pb, in_=P["projb"][:])
            ps = pss.tile([PRED, 2], FP, tag="sm", name="sm")
            for k in range(NB):
                nc.tensor.matmul(ps, lhsT=PW[k], rhs=h_bf[k], start=(k == 0),
                                 stop=(k == NB - 1))
            res = sing.tile([PRED, 2], FP)
            nc.vector.tensor_scalar(out=res, in0=ps, scalar1=pb, scalar2=None,
                                    op0=OP.add)
            nc.sync.dma_start(out=out_d[:, :], in_=res)

    nc.finalize()
    return nc


_CACHE = {}


def kernel(**inputs):
    w, xts, means, stdev = prep_host_inputs(inputs)
    if "nc" not in _CACHE:
        _CACHE["nc"] = build_program()
    nc = _CACHE["nc"]
    in_maps = []
    for b in range(8):
        m = dict(w)
        m["xT"] = xts[b]
        in_maps.append(m)
    rr = run_bass_kernel_spmd(nc, in_maps, list(range(8)))
    outs = []
    for b in range(8):
        o = np.asarray(rr.results[b]["out"], np.float32)     # [96, 2]
        o = o * stdev[b][None, :] + means[b][None, :]
        outs.append(o)
    return np.stack(outs)                                    # [8, 96, 2]


# revision 20
# speedup vs baseline: 1.2406x; 1.1111x over previous
"""Trainium2 Bass kernel for nn_Experiment6 (bi-mamba + MHA + FFN forecaster).

Sharding: data-parallel over batch (B=8) across 8 NeuronCores; all params
replicated. Inside each core: activations kept transposed [feature, time];
selective scan via DVE tensor_tensor_scan in n-major layout
[128 d-partitions, (n=16, t=512) free]; reverse-direction mamba handled with
reversed free-axis APs (no data reversal). Output depends only on positions
0,1 of the final sequence, so the last layer is pruned accordingly.
RevIN normalization and final rescale are host-side (exact fp32).
"""
import numpy as np

import concourse.bacc as bacc
import concourse.bass as bass
import concourse.tile as tile
from concourse import mybir
from concourse.bass_utils import run_bass_kernel_spmd

FP = mybir.dt.float32
BF = mybir.dt.bfloat16
AF = mybir.ActivationFunctionType
OP = mybir.AluOpType

L = 512
DM = 512
DS = 16
DF = 2048
DTR = 32
NH = 4
DH = 128
PRED = 96
EPS = 1e-5
NB = 4  # number of 128-partition blocks in DM


def _f(x):
    return np.ascontiguousarray(np.asarray(x, np.float32))


def _bf(x):
    import ml_dtypes
    return np.ascontiguousarray(np.asarray(x, np.float32).astype(ml_dtypes.bfloat16))


def prep_host_inputs(inputs):
    """Returns (shared weight map, per-core x maps, per-core (mean, std))."""
    w = {}
    w["Wp"] = _bf(inputs["Wp"])                                # [2, 512]
    w["bp"] = _f(inputs["bp"])
    s = 1.0 / np.sqrt(DH)
    w["Wq"] = _bf(_f(inputs["Wq"]) * s)
    w["bq"] = _f(_f(inputs["bq"]) * s)
    w["Wk"] = _bf(inputs["Wk"])
    w["bk"] = _f(inputs["bk"])
    w["Wv"] = _bf(inputs["Wv"])
    w["Wo"] = _bf(inputs["Wo"])
    # fold v-bias through Wo, plus bi (the empty-input branch bias)
    bo2 = _f(inputs["bo"]) + _f(inputs["bi"]) + _f(inputs["Wo"]).T @ _f(inputs["bv"])
    w["bo2"] = _f(bo2)
    for li in range(2):
        for dd in range(2):
            tag = f"{li}{dd}"
            w["Win" + tag] = _bf(inputs["m_Win"][li, dd])       # [512, 1024]
            w["convw" + tag] = _f(inputs["m_convw"][li, dd])    # [512, 2]
            w["convb" + tag] = _f(inputs["m_convb"][li, dd])    # [512]
            w["Wx" + tag] = _bf(inputs["m_Wx"][li, dd])         # [512, 64]
            w["Wdt" + tag] = _bf(inputs["m_Wdt"][li, dd])       # [32, 512]
            w["bdt" + tag] = _f(inputs["m_bdt"][li, dd])        # [512]
            w["Wout" + tag] = _bf(inputs["m_Wout"][li, dd])     # [512, 512]
    for li in range(2):
        w[f"ffW1_{li}"] = _bf(inputs["ff_W1"][li])              # [512, 2048]
        w[f"ffb1_{li}"] = _f(inputs["ff_b1"][li])
        w[f"ffW2_{li}"] = _bf(inputs["ff_W2"][li])              # [2048, 512]
        w[f"ffb2_{li}"] = _f(inputs["ff_b2"][li])
    w["projW"] = _bf(inputs["proj_W"])                          # [512, 96]
    w["projb"] = _f(inputs["proj_b"])

    x_enc = _f(inputs["x_enc"])                                 # [8, 512, 2]
    means = x_enc.mean(1, keepdims=True)                        # [8,1,2]
    xc = x_enc - means
    stdev = np.sqrt(xc.var(axis=1, keepdims=True) + 1e-5)
    xn = xc / stdev
    xts = [np.ascontiguousarray(xn[b].T) for b in range(8)]     # [2,512] each
    return w, xts, means[:, 0, :], stdev[:, 0, :]


def rev3(t):
    """Flat reversed AP over a contiguous [128, 16, 512] n-major tile: iterates
    (n desc, t desc) so each n-chain runs t-descending; block transitions are
    cut by the a=0 mask at t=511."""
    el = t.ap[-1][0]
    ntot = t.shape[1] * t.shape[2]
    return bass.AP(tensor=t.tensor, offset=t.offset + (ntot - 1) * el,
                   ap=[t.ap[0], [-el, ntot]])


def flat2(t, ntot):
    el = t.ap[-1][0]
    return bass.AP(tensor=t.tensor, offset=t.offset, ap=[t.ap[0], [el, ntot]])


def build_program():
    nc = bacc.Bacc()
    P = {}

    def par(name, shape, dt):
        P[name] = nc.declare_dram_parameter(name, list(shape), dt, isOutput=False)
        return P[name]

    par("xT", (2, L), FP)
    par("Wp", (2, DM), BF); par("bp", (DM,), FP)
    for nm in ("Wq", "Wk", "Wv", "Wo"):
        par(nm, (DM, DM), BF)
    par("bq", (DM,), FP); par("bk", (DM,), FP); par("bo2", (DM,), FP)
    for li in range(2):
        for dd in range(2):
            tg = f"{li}{dd}"
            par("Win" + tg, (DM, 2 * DM), BF)
            par("convw" + tg, (DM, 2), FP)
            par("convb" + tg, (DM,), FP)
            par("Wx" + tg, (DM, DTR + 2 * DS), BF)
            par("Wdt" + tg, (DTR, DM), BF)
            par("bdt" + tg, (DM,), FP)
            par("Wout" + tg, (DM, DM), BF)
    for li in range(2):
        par(f"ffW1_{li}", (DM, DF), BF); par(f"ffb1_{li}", (DF,), FP)
        par(f"ffW2_{li}", (DF, DM), BF); par(f"ffb2_{li}", (DM,), FP)
    par("projW", (DM, PRED), BF); par("projb", (PRED,), FP)
    out_d = nc.declare_dram_parameter("out", [PRED, 2], FP, isOutput=True)

    with tile.TileContext(nc) as tc:
        import contextlib
        ctx = contextlib.ExitStack()
        with ctx:
            sing = ctx.enter_context(tc.tile_pool(name="sing", bufs=1))
            scr = ctx.enter_context(tc.tile_pool(name="scr", bufs=2))
            scr1 = ctx.enter_context(tc.tile_pool(name="scr1", bufs=1))
            bigp = ctx.enter_context(tc.tile_pool(name="bigp", bufs=2))
            wpool = ctx.enter_context(tc.tile_pool(name="wp", bufs=1))
            big = ctx.enter_context(tc.tile_pool(name="big", bufs=1))
            psum = ctx.enter_context(tc.tile_pool(name="ps", bufs=2, space="PSUM"))
            psacc = ctx.enter_context(tc.tile_pool(name="psacc", bufs=4, space="PSUM"))
            pss = ctx.enter_context(tc.tile_pool(name="pss", bufs=2, space="PSUM"))
            dram = ctx.enter_context(tc.tile_pool(name="dr", bufs=1, space="DRAM"))

            def vec(name, n=DM, dt=FP):
                """load a DRAM vector as NB [128,1] bias tiles"""
                ts = []
                for g in range(n // 128):
                    t = sing.tile([128, 1], dt, tag=f"v_{name}_{g}", name=f"v_{name}_{g}")
                    nc.sync.dma_start(out=t, in_=P[name][g * 128:(g + 1) * 128])
                    ts.append(t)
                return ts

            def wload(name, rows, cols, tag=None, dt=BF):
                """load weight [rows, cols] as rows//128 k-tiles"""
                ts = []
                nk = max(1, rows // 128)
                kr = rows // nk
                for k in range(nk):
                    t = wpool.tile([kr, cols], dt, tag=(tag or name) + f"_{k}")
                    nc.sync.dma_start(out=t, in_=P[name][k * kr:(k + 1) * kr, :])
                    ts.append(t)
                return ts

            ones_c = sing.tile([128, 1], FP)
            nc.vector.memset(ones_c, 1.0)
            ones_r = sing.tile([1, 128], FP)
            nc.vector.memset(ones_r, 1.0)
            eps_t = sing.tile([1, 1], FP)
            nc.vector.memset(eps_t, EPS)

            # ---- embed: ppT = Wp^T @ xT + bp ----
            xT = sing.tile([2, L], FP)
            nc.sync.dma_start(out=xT, in_=P["xT"][:, :])
            xTb = sing.tile([2, L], BF)
            nc.vector.tensor_copy(out=xTb, in_=xT)
            Wp_t = wload("Wp", 2, DM, tag="wp512x")  # [2, 512] single tile (rows<128)
            bp_t = vec("bp")
            pp_bf = [sing.tile([128, L], BF, tag=f"ppbf{g}", name=f"ppbf{g}") for g in range(NB)]
            for g in range(NB):
                ps = psum.tile([128, L], FP, tag="tr", name="tr")
                nc.tensor.matmul(ps, lhsT=Wp_t[0][:, g * 128:(g + 1) * 128],
                                 rhs=xTb, start=True, stop=True)
                nc.vector.tensor_scalar(out=pp_bf[g], in0=ps, scalar1=bp_t[g],
                                        scalar2=None, op0=OP.add)

            # ---- MHA ----
            def proj_T(wname, bias_ts, outdt=BF):
                """outT[do, t] = W^T @ pp (+bias): returns NB tiles"""
                Wt = wload(wname, DM, DM, tag="w512")
                outs = []
                for m in range(NB):
                    ps = psum.tile([128, L], FP, tag="tr", name="tr")
                    for k in range(NB):
                        nc.tensor.matmul(ps, lhsT=Wt[k][:, m * 128:(m + 1) * 128],
                                         rhs=pp_bf[k], start=(k == 0),
                                         stop=(k == NB - 1))
                    o = sing.tile([128, L], outdt, tag=f"{wname}_o{m}", name=f"{wname}_o{m}")
                    if bias_ts is None:
                        nc.scalar.copy(out=o, in_=ps)
                    else:
                        nc.vector.tensor_scalar(out=o, in0=ps, scalar1=bias_ts[m],
                                                scalar2=None, op0=OP.add)
                    outs.append(o)
                return outs

            qT = proj_T("Wq", vec("bq"))
            kT = proj_T("Wk", vec("bk"))
            # V in natural layout: V[t, d] = pp[t, :] @ Wv
            Wv_t = wload("Wv", DM, DM, tag="w512")
            Vn = []
            for m in range(NB):  # m indexes t-blocks
                ps = psum.tile([128, L], FP, tag="tr", name="tr")
                for k in range(NB):
                    nc.tensor.matmul(ps, lhsT=pp_bf[k][:, m * 128:(m + 1) * 128],
                                     rhs=Wv_t[k], start=(k == 0), stop=(k == NB - 1))
                o = sing.tile([128, L], BF, tag=f"vn{m}", name=f"vn{m}")
                nc.scalar.copy(out=o, in_=ps)
                Vn.append(o)

            oT = [sing.tile([128, L], BF, tag=f"oT{h}", name=f"oT{h}") for h in range(NH)]
            for h in range(NH):
                # ST[m, l] = K_h^T Q_h ; E = exp(ST); denom = ones^T E
                E_h = []
                dn = pss.tile([1, L], FP, tag="sm", name="sm")
                for mb in range(NB):
                    ps = psum.tile([128, L], FP, tag="tr", name="tr")
                    nc.tensor.matmul(ps, lhsT=kT[h][:, mb * 128:(mb + 1) * 128],
                                     rhs=qT[h], start=True, stop=True)
                    e = scr1.tile([128, L], BF, tag=f"eh{mb}", name=f"eh{mb}")
                    nc.scalar.activation(out=e, in_=ps, func=AF.Exp)
                    E_h.append(e)
                ob = scr.tile([1, 128], BF, tag="onesbf", name="onesbf")
                nc.vector.tensor_copy(out=ob, in_=ones_r)
                oc = scr.tile([128, 1], BF, tag="onescbf", name="onescbf")
                nc.vector.tensor_copy(out=oc, in_=ones_c)
                for mb in range(NB):
                    nc.tensor.matmul(dn, lhsT=oc, rhs=E_h[mb],
                                     start=(mb == 0), stop=(mb == NB - 1))
                rinv = scr.tile([1, L], FP, tag="rinv", name="rinv")
                nc.vector.reciprocal_approx_fast(out=rinv, in_=dn)
                rb = scr.tile([1, L], BF, tag="rb", name="rb")
                nc.vector.tensor_copy(out=rb, in_=rinv)
                rrep = psum.tile([128, L], FP, tag="tr", name="tr")
                nc.tensor.matmul(rrep, lhsT=ob, rhs=rb, start=True, stop=True)
                rrs = scr.tile([128, L], FP, tag="rrs", name="rrs")
                nc.scalar.copy(out=rrs, in_=rrep)
                # AV: OT_h = sum_m V[m, dh] E[m, l]
                av = psum.tile([128, L], FP, tag="tr", name="tr")
                for mb in range(NB):
                    nc.tensor.matmul(av, lhsT=Vn[mb][:, h * 128:(h + 1) * 128],
                                     rhs=E_h[mb], start=(mb == 0),
                                     stop=(mb == NB - 1))
                nc.vector.tensor_tensor(out=oT[h], in0=av, in1=rrs, op=OP.mult)

            bo2_t = vec("bo2")
            Wo_t = wload("Wo", DM, DM, tag="w512")
            hT = [sing.tile([128, L], FP, tag=f"hT{g}", name=f"hT{g}") for g in range(NB)]
            for m in range(NB):
                ps = psum.tile([128, L], FP, tag="tr", name="tr")
                for k in range(NB):
                    nc.tensor.matmul(ps, lhsT=Wo_t[k][:, m * 128:(m + 1) * 128],
                                     rhs=oT[k], start=(k == 0), stop=(k == NB - 1))
                nc.vector.tensor_scalar(out=hT[m], in0=ps, scalar1=bo2_t[m],
                                        scalar2=None, op0=OP.add)

            # ---- persistent mamba tiles ----
            NH2 = DS // 4
            B_rep = big.tile([128, NH2, L], BF, tag="Brep", name="Brep")
            C_rep = big.tile([128, NH2, L], BF, tag="Crep", name="Crep")
            dbl_dram = dram.tile([64, L], BF, tag="dbldram", name="dbldram")

            def emit_mamba(li, dd, h_bf, last):
                tg = f"{li}{dd}"
                rev = dd == 1
                Tn = 2 if (last and not rev) else L
                # Win matmuls: x-half always full T (rev) or Tn; z-half Tn2
                def win_half(co):
                    ts = []
                    for k in range(NB):
                        t = wpool.tile([128, DM], BF, tag=f"win_{k}",
                                       name=f"win_{k}")
                        nc.sync.dma_start(
                            out=t, in_=P["Win" + tg][k * 128:(k + 1) * 128,
                                                     co:co + DM])
                        ts.append(t)
                    return ts

                Win_t = win_half(0)
                Tx = L if not last or rev else 3
                xcpre = []
                for m in range(NB):
                    ps = psacc.tile([128, L], FP, tag="acc", name="acc")
                    for k in range(NB):
                        nc.tensor.matmul(ps[:, 0:Tx],
                                         lhsT=Win_t[k][:, m * 128:(m + 1) * 128],
                                         rhs=h_bf[k][:, 0:Tx], start=(k == 0),
                                         stop=(k == NB - 1))
                    xcpre.append(ps)
                Tz = 2 if last else L
                Win_z = win_half(DM)
                zsil = []
                for m in range(NB):
                    ps = psum.tile([128, L], FP, tag="tr", name="tr")
                    for k in range(NB):
                        nc.tensor.matmul(
                            ps[:, 0:Tz],
                            lhsT=Win_z[k][:, m * 128:(m + 1) * 128],
                            rhs=h_bf[k][:, 0:Tz], start=(k == 0), stop=(k == NB - 1))
                    o = sing.tile([128, L], BF, tag=f"zsil{m}", name=f"zsil{m}")
                    nc.scalar.activation(out=o[:, 0:Tz], in_=ps[:, 0:Tz], func=AF.Silu)
                    zsil.append(o)

                convw = P["convw" + tg]
                w0 = [sing.tile([128, 1], FP, tag=f"w0_{g}", name=f"w0_{g}") for g in range(NB)]
                w1 = [sing.tile([128, 1], FP, tag=f"w1_{g}", name=f"w1_{g}") for g in range(NB)]
                for g in range(NB):
                    nc.sync.dma_start(out=w0[g],
                                      in_=convw[g * 128:(g + 1) * 128, 0:1])
                    nc.sync.dma_start(out=w1[g],
                                      in_=convw[g * 128:(g + 1) * 128, 1:2])
                cb_t = vec("convb" + tg)
                xcT = [sing.tile([128, L], BF, tag=f"xcT{g}", name=f"xcT{g}") for g in range(NB)]
                Tc = Tx if (last and not rev) else L
                for g in range(NB):
                    t1 = scr.tile([128, L], FP, tag="convt1", name="convt1")
                    nc.vector.tensor_scalar(out=t1[:, 0:Tc], in0=xcpre[g][:, 0:Tc],
                                            scalar1=w1[g], scalar2=cb_t[g],
                                            op0=OP.mult, op1=OP.add)
                    c2 = scr.tile([128, L], FP, tag="convt2", name="convt2")
                    if not rev:
                        nc.vector.scalar_tensor_tensor(
                            out=c2[:, 1:Tc], in0=xcpre[g][:, 0:Tc - 1],
                            scalar=w0[g], in1=t1[:, 1:Tc], op0=OP.mult, op1=OP.add)
                        nc.vector.tensor_copy(out=c2[:, 0:1], in_=t1[:, 0:1])
                    else:
                        nc.vector.scalar_tensor_tensor(
                            out=c2[:, 0:Tc - 1], in0=xcpre[g][:, 1:Tc],
                            scalar=w0[g], in1=t1[:, 0:Tc - 1], op0=OP.mult,
                            op1=OP.add)
                        nc.vector.tensor_copy(out=c2[:, Tc - 1:Tc],
                                              in_=t1[:, Tc - 1:Tc])
                    nc.scalar.activation(out=xcT[g][:, 0:Tn], in_=c2[:, 0:Tn],
                                         func=AF.Silu)

                # dbl = Wx^T @ xc  [64, Tn]
                Wx_t = wload("Wx" + tg, DM, 64, tag="wx")
                psd = pss.tile([64, L], FP, tag="sm", name="sm")
                for k in range(NB):
                    nc.tensor.matmul(psd[:, 0:Tn], lhsT=Wx_t[k],
                                     rhs=xcT[k][:, 0:Tn],
                                     start=(k == 0), stop=(k == NB - 1))
                dblT = scr.tile([64, L], FP, tag="dblT", name="dblT")
                nc.scalar.copy(out=dblT[:, 0:Tn], in_=psd[:, 0:Tn])
                dbl_bf = scr.tile([64, L], BF, tag="dblbf", name="dblbf")
                nc.vector.tensor_copy(out=dbl_bf[:, 0:Tn], in_=dblT[:, 0:Tn])
                nc.sync.dma_start(out=dbl_dram[:, 0:Tn], in_=dbl_bf[:, 0:Tn])
                dtraw = scr.tile([DTR, L], BF, tag="dtraw", name="dtraw")
                nc.vector.tensor_copy(out=dtraw[:, 0:Tn], in_=dblT[0:DTR, 0:Tn])

                # dt = softplus(Wdt^T @ dtraw + bdt)
                Wdt_t = wload("Wdt" + tg, DTR, DM, tag="wdt512")
                bdt_t = vec("bdt" + tg)
                dtT = [sing.tile([128, L], FP, tag=f"dtT{g}", name=f"dtT{g}") for g in range(NB)]
                duT = [sing.tile([128, L], BF, tag=f"duT{g}", name=f"duT{g}") for g in range(NB)]
                for g in range(NB):
                    ps = psum.tile([128, L], FP, tag="tr", name="tr")
                    nc.tensor.matmul(ps[:, 0:Tn],
                                     lhsT=Wdt_t[0][:, g * 128:(g + 1) * 128],
                                     rhs=dtraw[:, 0:Tn], start=True, stop=True)
                    esp = scr.tile([128, L], FP, tag="esp", name="esp")
                    nc.scalar.activation(out=esp[:, 0:Tn], in_=ps[:, 0:Tn],
                                         func=AF.Exp, bias=bdt_t[g])
                    nc.scalar.activation(out=dtT[g][:, 0:Tn], in_=esp[:, 0:Tn],
                                         func=AF.Ln, bias=1.0)
                    nc.vector.tensor_tensor(out=duT[g][:, 0:Tn],
                                            in0=dtT[g][:, 0:Tn],
                                            in1=xcT[g][:, 0:Tn], op=OP.mult)

                dap = dbl_dram[:, :]
                el = dap.ap[-1][0]

                yT = [sing.tile([128, L], FP, tag=f"yT{g}", name=f"yT{g}") for g in range(NB)]
                small = last and not rev
                yT = None
                yTl = [sing.tile([128, L], FP, tag=f"yT{g}", name=f"yT{g}")
                       for g in range(NB)]
                yt2 = scr.tile([128, L], FP, tag="yt2", name="yt2")
                for nh in range(4):
                    # broadcast B/C halves for this mamba
                    def bcast(dst, row0):
                        src = bass.AP(tensor=dap.tensor,
                                      offset=dap.offset + row0 * L * el,
                                      ap=[[0, 128], [L * el, NH2], [el, Tn]])
                        nc.sync.dma_start(out=dst[:, :, 0:Tn], in_=src)
                    bcast(B_rep, DTR + nh * NH2)
                    if not last:
                        bcast(C_rep, DTR + DS + nh * NH2)
                    for g in range(NB):
                        if small:
                            A2s = scr.tile([128, NH2, 2], BF, tag="A2s", name="A2s")
                            dBu2s = scr.tile([128, NH2, 2], BF, tag="dBu2s",
                                             name="dBu2s")
                            At, dBt, Ht2 = A2s, dBu2s, dBu2s
                            AL = 2
                        else:
                            A_blk = bigp.tile([128, NH2, L], BF, tag="Ablk",
                                              name="Ablk")
                            dBu_blk = bigp.tile([128, NH2, L], BF, tag="dBublk",
                                                name="dBublk")
                            At, dBt, Ht2 = A_blk, dBu_blk, dBu_blk
                            AL = L
                        for n in range(NH2):
                            nc.scalar.activation(out=At[:, n, 0:Tn],
                                                 in_=dtT[g][:, 0:Tn], func=AF.Exp,
                                                 scale=-float(nh * NH2 + n + 1))
                        ael = At.ap[-1][0]
                        t0 = 0 if not rev else Tn - 1
                        mask = bass.AP(tensor=At.tensor,
                                       offset=At.offset + t0 * ael,
                                       ap=[At.ap[0], [AL * ael, NH2], [ael, 1]])
                        nc.vector.memset(mask, 0.0)
                        del_ = duT[g].ap[-1][0]
                        du_s0 = bass.AP(tensor=duT[g].tensor, offset=duT[g].offset,
                                        ap=[duT[g].ap[0], [0, NH2], [del_, Tn]])
                        nc.vector.tensor_tensor(out=dBt[:, :, 0:Tn], in0=du_s0,
                                                in1=B_rep[:, :, 0:Tn], op=OP.mult)
                        if not small:
                            if not rev:
                                nc.vector.tensor_tensor_scan(
                                    out=flat2(dBu_blk, NH2 * L),
                                    data0=flat2(A_blk, NH2 * L),
                                    data1=flat2(dBu_blk, NH2 * L), initial=0.0,
                                    op0=OP.mult, op1=OP.add)
                            else:
                                nc.vector.tensor_tensor_scan(
                                    out=rev3(dBu_blk), data0=rev3(A_blk),
                                    data1=rev3(dBu_blk), initial=0.0,
                                    op0=OP.mult, op1=OP.add)
                        else:
                            nc.vector.tensor_tensor_scan(
                                out=flat2(dBu2s, NH2 * 2), data0=flat2(A2s, NH2 * 2),
                                data1=flat2(dBu2s, NH2 * 2), initial=0.0,
                                op0=OP.mult, op1=OP.add)
                        ytarget = yTl[g] if nh == 0 else yt2
                        if not last:
                            ych = Ht2  # in-place: H *= C_rep
                            nc.vector.tensor_tensor(out=ych, in0=Ht2, in1=C_rep,
                                                    op=OP.mult)
                            # n-reduce as bf16 2x add tree over contiguous slices
                            nc.vector.tensor_tensor(out=ych[:, 0, :],
                                                    in0=ych[:, 0, :],
                                                    in1=ych[:, 1, :], op=OP.add)
                            nc.vector.tensor_tensor(out=ych[:, 2, :],
                                                    in0=ych[:, 2, :],
                                                    in1=ych[:, 3, :], op=OP.add)
                            nc.vector.tensor_tensor(out=ytarget, in0=ych[:, 0, :],
                                                    in1=ych[:, 2, :], op=OP.add)
                        else:
                            if small:
                                h_sl = Ht2[:, :, :]
                            else:
                                hel = Ht2.ap[-1][0]
                                h_sl = bass.AP(tensor=Ht2.tensor, offset=Ht2.offset,
                                               ap=[Ht2.ap[0], [L * hel, NH2],
                                                   [hel, 2]])
                            c2t = scr.tile([128, NH2, 2], BF, tag="c2t", name="c2t")
                            csrc = bass.AP(
                                tensor=dap.tensor,
                                offset=dap.offset + (DTR + DS + nh * NH2) * L * el,
                                ap=[[0, 128], [L * el, NH2], [el, 2]])
                            nc.sync.dma_start(out=c2t, in_=csrc)
                            tmp = scr.tile([128, NH2, 2], BF, tag="ychs",
                                           name="ychs")
                            nc.vector.tensor_tensor(out=tmp, in0=h_sl, in1=c2t,
                                                    op=OP.mult)
                            tel = tmp.ap[-1][0]
                            red_in = bass.AP(tensor=tmp.tensor, offset=tmp.offset,
                                             ap=[tmp.ap[0], [tel, 2],
                                                 [2 * tel, NH2]])
                            nc.vector.tensor_reduce(out=ytarget[:, 0:2],
                                                    in_=red_in,
                                                    axis=mybir.AxisListType.X,
                                                    op=OP.add)
                        if nh > 0:
                            Ty = 2 if last else L
                            nc.vector.tensor_tensor(out=yTl[g][:, 0:Ty],
                                                    in0=yTl[g][:, 0:Ty],
                                                    in1=yt2[:, 0:Ty], op=OP.add)
                yT = yTl

                # gate: g = (y + xc) * zsil  -> bf16
                gT = [scr.tile([128, L], BF, tag=f"gT{g}", name=f"gT{g}") for g in range(NB)]
                Tg = 2 if last else L
                for g in range(NB):
                    nc.vector.tensor_tensor(out=yT[g][:, 0:Tg], in0=yT[g][:, 0:Tg],
                                            in1=xcT[g][:, 0:Tg], op=OP.add)
                    nc.vector.tensor_tensor(out=gT[g][:, 0:Tg], in0=yT[g][:, 0:Tg],
                                            in1=zsil[g][:, 0:Tg], op=OP.mult)
                return gT, Tg

            def emit_layer(li):
                last = li == 1
                h_bf = [scr1.tile([128, L], BF, tag=f"hbf{g}", name=f"hbf{g}") for g in range(NB)]
                for g in range(NB):
                    nc.vector.tensor_copy(out=h_bf[g], in_=hT[g])
                g_f, Tg_f = emit_mamba(li, 0, h_bf, last)
                g_r, Tg_r = emit_mamba(li, 1, h_bf, last)
                Tm = 2 if last else L
                pso = [psacc.tile([128, L], FP, tag="acc", name="acc")
                       for _ in range(NB)]
                for dd, gg in ((0, g_f), (1, g_r)):
                    Wd = wload(f"Wout{li}{dd}", DM, DM, tag="wout")
                    for m in range(NB):
                        for k in range(NB):
                            nc.tensor.matmul(
                                pso[m][:, 0:Tm],
                                lhsT=Wd[k][:, m * 128:(m + 1) * 128],
                                rhs=gg[k][:, 0:Tm], start=(dd == 0 and k == 0),
                                stop=(dd == 1 and k == NB - 1))
                for m in range(NB):
                    nc.vector.tensor_tensor(out=hT[m][:, 0:Tm],
                                            in0=hT[m][:, 0:Tm], in1=pso[m][:, 0:Tm],
                                            op=OP.add)
                ln_inplace(Tm)
                ffn(li, Tm, last)

            def ln_inplace(T):
                """layernorm over d (partitions) of hT[:, 0:T], in place."""
                psm = pss.tile([1, L], FP, tag="sm", name="sm")
                psq = pss.tile([1, L], FP, tag="sm", name="sm")
                for g in range(NB):
                    sq = scr.tile([128, L], FP, tag="lnsq", name="lnsq")
                    nc.scalar.activation(out=sq[:, 0:T], in_=hT[g][:, 0:T],
                                         func=AF.Square)
                    nc.tensor.matmul(psm[:, 0:T], lhsT=ones_c, rhs=hT[g][:, 0:T],
                                     start=(g == 0), stop=(g == NB - 1))
                    nc.tensor.matmul(psq[:, 0:T], lhsT=ones_c, rhs=sq[:, 0:T],
                                     start=(g == 0), stop=(g == NB - 1))
                mean = scr.tile([1, L], FP, tag="lnmean", name="lnmean")
                nc.vector.tensor_scalar(out=mean[:, 0:T], in0=psm[:, 0:T],
                                        scalar1=1.0 / DM, scalar2=None, op0=OP.mult)
                m2 = scr.tile([1, L], FP, tag="lnm2", name="lnm2")
                nc.vector.tensor_tensor(out=m2[:, 0:T], in0=mean[:, 0:T],
                                        in1=mean[:, 0:T], op=OP.mult)
                var = scr.tile([1, L], FP, tag="lnvar", name="lnvar")
                nc.vector.scalar_tensor_tensor(out=var[:, 0:T], in0=psq[:, 0:T],
                                               scalar=1.0 / DM, in1=m2[:, 0:T],
                                               op0=OP.mult, op1=OP.subtract)
                sd = scr.tile([1, L], FP, tag="lnsd", name="lnsd")
                nc.scalar.activation(out=sd[:, 0:T], in_=var[:, 0:T],
                                     func=AF.Sqrt, bias=eps_t)
                rinv = scr.tile([1, L], FP, tag="lnrinv", name="lnrinv")
                nc.vector.reciprocal_approx_fast(out=rinv[:, 0:T], in_=sd[:, 0:T])
                mrep = psum.tile([128, L], FP, tag="tr", name="tr")
                nc.tensor.matmul(mrep[:, 0:T], lhsT=ones_r, rhs=mean[:, 0:T],
                                 start=True, stop=True)
                rrep = psum.tile([128, L], FP, tag="tr", name="tr")
                nc.tensor.matmul(rrep[:, 0:T], lhsT=ones_r, rhs=rinv[:, 0:T],
                                 start=True, stop=True)
                mrs = scr.tile([128, L], FP, tag="lnmrs", name="lnmrs")
                nc.scalar.copy(out=mrs[:, 0:T], in_=mrep[:, 0:T])
                rrs = scr.tile([128, L], FP, tag="lnrrs", name="lnrrs")
                nc.scalar.copy(out=rrs[:, 0:T], in_=rrep[:, 0:T])
                for g in range(NB):
                    c = scr.tile([128, L], FP, tag="lnc", name="lnc")
                    nc.vector.tensor_tensor(out=c[:, 0:T], in0=hT[g][:, 0:T],
                                            in1=mrs[:, 0:T], op=OP.subtract)
                    nc.vector.tensor_tensor(out=hT[g][:, 0:T], in0=c[:, 0:T],
                                            in1=rrs[:, 0:T], op=OP.mult)

            def ffn(li, T, last):
                h_bf = [scr1.tile([128, L], BF, tag=f"fhbf{g}", name=f"fhbf{g}") for g in range(NB)]
                for g in range(NB):
                    nc.vector.tensor_copy(out=h_bf[g][:, 0:T], in_=hT[g][:, 0:T])
                b1 = vec(f"ffb1_{li}", DF)
                b2 = vec(f"ffb2_{li}")
                pso = [psacc.tile([128, L], FP, tag="acc", name="acc")
                       for _ in range(NB)]
                for half in range(4):
                    W1 = []
                    for k in range(NB):
                        t = wpool.tile([128, DF // 4], BF, tag=f"ffw1_{k}",
                                       name=f"ffw1_{k}")
                        nc.sync.dma_start(
                            out=t, in_=P[f"ffW1_{li}"][k * 128:(k + 1) * 128,
                                                       half * (DF // 4):
                                                       (half + 1) * (DF // 4)])
                        W1.append(t)
                    yb = [scr1.tile([128, L], BF, tag=f"ffyb{k}", name=f"ffyb{k}")
                          for k in range(4)]
                    for k8 in range(4):
                        m = half * 4 + k8
                        ps = psum.tile([128, L], FP, tag="tr", name="tr")
                        for k in range(NB):
                            nc.tensor.matmul(ps[:, 0:T],
                                             lhsT=W1[k][:, k8 * 128:(k8 + 1) * 128],
                                             rhs=h_bf[k][:, 0:T], start=(k == 0),
                                             stop=(k == NB - 1))
                        nc.scalar.activation(out=yb[k8][:, 0:T], in_=ps[:, 0:T],
                                             func=AF.Relu, bias=b1[m])
                    W2h = []
                    for k8 in range(4):
                        t = wpool.tile([128, DM], BF, tag=f"ffw2_{k8}",
                                       name=f"ffw2_{k8}")
                        r0 = (half * 4 + k8) * 128
                        nc.sync.dma_start(out=t,
                                          in_=P[f"ffW2_{li}"][r0:r0 + 128, :])
                        W2h.append(t)
                    for m in range(NB):
                        for k8 in range(4):
                            nc.tensor.matmul(
                                pso[m][:, 0:T],
                                lhsT=W2h[k8][:, m * 128:(m + 1) * 128],
                                rhs=yb[k8][:, 0:T], start=(half == 0 and k8 == 0),
                                stop=(half == 3 and k8 == 3))
                for m in range(NB):
                    nc.vector.scalar_tensor_tensor(out=hT[m][:, 0:T],
                                                   in0=pso[m][:, 0:T], scalar=b2[m],
                                                   in1=hT[m][:, 0:T], op0=OP.add,
                                                   op1=OP.add)
                ln_inplace(T)

            emit_layer(0)
            emit_layer(1)

            # final projection at positions 0,1
            h_bf = [scr.tile([128, 2], BF, tag=f"pjb{g}", name=f"pjb{g}") for g in range(NB)]
            for g in range(NB):
                nc.vector.tensor_copy(out=h_bf[g], in_=hT[g][:, 0:2])
            PW = wload("projW", DM, PRED, tag="w512")
            pb = sing.tile([PRED, 1], FP)
            nc.sync.dma_start(out=pb, in_=P["projb"][:])
            ps = pss.tile([PRED, 2], FP, tag="sm", name="sm")
            for k in range(NB):
                nc.tensor.matmul(ps, lhsT=PW[k], rhs=h_bf[k], start=(k == 0),
                                 stop=(k == NB - 1))
            res = sing.tile([PRED, 2], FP)
            nc.vector.tensor_scalar(out=res, in0=ps, scalar1=pb, scalar2=None,
                                    op0=OP.add)
            nc.sync.dma_start(out=out_d[:, :], in_=res)

    nc.finalize()
    return nc


_CACHE = {}


def kernel(**inputs):
    w, xts, means, stdev = prep_host_inputs(inputs)
    if "nc" not in _CACHE:
        _CACHE["nc"] = build_program()
    nc = _CACHE["nc"]
    in_maps = []
    for b in range(8):
        m = dict(w)
        m["xT"] = xts[b]
        in_maps.append(m)
    rr = run_bass_kernel_spmd(nc, in_maps, list(range(8)))
    outs = []
    for b in range(8):
        o = np.asarray(rr.results[b]["out"], np.float32)     # [96, 2]
        o = o * stdev[b][None, :] + means[b][None, :]
        outs.append(o)
    return np.stack(outs)                                    # [8, 96, 2]


# revision 21
# speedup vs baseline: 1.2659x; 1.0205x over previous
"""Trainium2 Bass kernel for nn_Experiment6 (bi-mamba + MHA + FFN forecaster).

Sharding: data-parallel over batch (B=8) across 8 NeuronCores; all params
replicated. Inside each core: activations kept transposed [feature, time];
selective scan via DVE tensor_tensor_scan in n-major layout
[128 d-partitions, (n=16, t=512) free]; reverse-direction mamba handled with
reversed free-axis APs (no data reversal). Output depends only on positions
0,1 of the final sequence, so the last layer is pruned accordingly.
RevIN normalization and final rescale are host-side (exact fp32).
"""
import numpy as np

import concourse.bacc as bacc
import concourse.bass as bass
import concourse.tile as tile
from concourse import mybir
from concourse.bass_utils import run_bass_kernel_spmd

FP = mybir.dt.float32
BF = mybir.dt.bfloat16
AF = mybir.ActivationFunctionType
OP = mybir.AluOpType

L = 512
DM = 512
DS = 16
DF = 2048
DTR = 32
NH = 4
DH = 128
PRED = 96
EPS = 1e-5
NB = 4  # number of 128-partition blocks in DM


def _f(x):
    return np.ascontiguousarray(np.asarray(x, np.float32))


def _bf(x):
    import ml_dtypes
    return np.ascontiguousarray(np.asarray(x, np.float32).astype(ml_dtypes.bfloat16))


def prep_host_inputs(inputs):
    """Returns (shared weight map, per-core x maps, per-core (mean, std))."""
    w = {}
    w["Wp"] = _bf(inputs["Wp"])                                # [2, 512]
    w["bp"] = _f(inputs["bp"])
    s = 1.0 / np.sqrt(DH)
    w["Wq"] = _bf(_f(inputs["Wq"]) * s)
    w["bq"] = _f(_f(inputs["bq"]) * s)
    w["Wk"] = _bf(inputs["Wk"])
    w["bk"] = _f(inputs["bk"])
    w["Wv"] = _bf(inputs["Wv"])
    w["Wo"] = _bf(inputs["Wo"])
    # fold v-bias through Wo, plus bi (the empty-input branch bias)
    bo2 = _f(inputs["bo"]) + _f(inputs["bi"]) + _f(inputs["Wo"]).T @ _f(inputs["bv"])
    w["bo2"] = _f(bo2)
    for li in range(2):
        for dd in range(2):
            tag = f"{li}{dd}"
            w["Win" + tag] = _bf(inputs["m_Win"][li, dd])       # [512, 1024]
            w["convw" + tag] = _f(inputs["m_convw"][li, dd])    # [512, 2]
            w["convb" + tag] = _f(inputs["m_convb"][li, dd])    # [512]
            w["Wx" + tag] = _bf(inputs["m_Wx"][li, dd])         # [512, 64]
            w["Wdt" + tag] = _bf(inputs["m_Wdt"][li, dd])       # [32, 512]
            w["bdt" + tag] = _f(inputs["m_bdt"][li, dd])        # [512]
            w["Wout" + tag] = _bf(inputs["m_Wout"][li, dd])     # [512, 512]
    for li in range(2):
        w[f"ffW1_{li}"] = _bf(inputs["ff_W1"][li])              # [512, 2048]
        w[f"ffb1_{li}"] = _f(inputs["ff_b1"][li])
        w[f"ffW2_{li}"] = _bf(inputs["ff_W2"][li])              # [2048, 512]
        w[f"ffb2_{li}"] = _f(inputs["ff_b2"][li])
    w["projW"] = _bf(inputs["proj_W"])                          # [512, 96]
    w["projb"] = _f(inputs["proj_b"])

    x_enc = _f(inputs["x_enc"])                                 # [8, 512, 2]
    means = x_enc.mean(1, keepdims=True)                        # [8,1,2]
    xc = x_enc - means
    stdev = np.sqrt(xc.var(axis=1, keepdims=True) + 1e-5)
    xn = xc / stdev
    xts = [np.ascontiguousarray(xn[b].T) for b in range(8)]     # [2,512] each
    return w, xts, means[:, 0, :], stdev[:, 0, :]


def rev3(t):
    """Flat reversed AP over a contiguous [128, 16, 512] n-major tile: iterates
    (n desc, t desc) so each n-chain runs t-descending; block transitions are
    cut by the a=0 mask at t=511."""
    el = t.ap[-1][0]
    ntot = t.shape[1] * t.shape[2]
    return bass.AP(tensor=t.tensor, offset=t.offset + (ntot - 1) * el,
                   ap=[t.ap[0], [-el, ntot]])


def flat2(t, ntot):
    el = t.ap[-1][0]
    return bass.AP(tensor=t.tensor, offset=t.offset, ap=[t.ap[0], [el, ntot]])


def build_program():
    nc = bacc.Bacc()
    P = {}

    def par(name, shape, dt):
        P[name] = nc.declare_dram_parameter(name, list(shape), dt, isOutput=False)
        return P[name]

    par("xT", (2, L), FP)
    par("Wp", (2, DM), BF); par("bp", (DM,), FP)
    for nm in ("Wq", "Wk", "Wv", "Wo"):
        par(nm, (DM, DM), BF)
    par("bq", (DM,), FP); par("bk", (DM,), FP); par("bo2", (DM,), FP)
    for li in range(2):
        for dd in range(2):
            tg = f"{li}{dd}"
            par("Win" + tg, (DM, 2 * DM), BF)
            par("convw" + tg, (DM, 2), FP)
            par("convb" + tg, (DM,), FP)
            par("Wx" + tg, (DM, DTR + 2 * DS), BF)
            par("Wdt" + tg, (DTR, DM), BF)
            par("bdt" + tg, (DM,), FP)
            par("Wout" + tg, (DM, DM), BF)
    for li in range(2):
        par(f"ffW1_{li}", (DM, DF), BF); par(f"ffb1_{li}", (DF,), FP)
        par(f"ffW2_{li}", (DF, DM), BF); par(f"ffb2_{li}", (DM,), FP)
    par("projW", (DM, PRED), BF); par("projb", (PRED,), FP)
    out_d = nc.declare_dram_parameter("out", [PRED, 2], FP, isOutput=True)

    with tile.TileContext(nc) as tc:
        import contextlib
        ctx = contextlib.ExitStack()
        with ctx:
            sing = ctx.enter_context(tc.tile_pool(name="sing", bufs=1))
            scr = ctx.enter_context(tc.tile_pool(name="scr", bufs=2))
            scr1 = ctx.enter_context(tc.tile_pool(name="scr1", bufs=1))
            bigp = ctx.enter_context(tc.tile_pool(name="bigp", bufs=2))
            wpool = ctx.enter_context(tc.tile_pool(name="wp", bufs=1))
            big = ctx.enter_context(tc.tile_pool(name="big", bufs=1))
            psum = ctx.enter_context(tc.tile_pool(name="ps", bufs=2, space="PSUM"))
            psacc = ctx.enter_context(tc.tile_pool(name="psacc", bufs=4, space="PSUM"))
            pss = ctx.enter_context(tc.tile_pool(name="pss", bufs=2, space="PSUM"))
            dram = ctx.enter_context(tc.tile_pool(name="dr", bufs=1, space="DRAM"))

            def vec(name, n=DM, dt=FP):
                """load a DRAM vector as NB [128,1] bias tiles"""
                ts = []
                for g in range(n // 128):
                    t = sing.tile([128, 1], dt, tag=f"v_{name}_{g}", name=f"v_{name}_{g}")
                    nc.sync.dma_start(out=t, in_=P[name][g * 128:(g + 1) * 128])
                    ts.append(t)
                return ts

            def wload(name, rows, cols, tag=None, dt=BF):
                """load weight [rows, cols] as rows//128 k-tiles"""
                ts = []
                nk = max(1, rows // 128)
                kr = rows // nk
                for k in range(nk):
                    t = wpool.tile([kr, cols], dt, tag=(tag or name) + f"_{k}")
                    nc.sync.dma_start(out=t, in_=P[name][k * kr:(k + 1) * kr, :])
                    ts.append(t)
                return ts

            ones_c = sing.tile([128, 1], FP)
            nc.vector.memset(ones_c, 1.0)
            ones_r = sing.tile([1, 128], FP)
            nc.vector.memset(ones_r, 1.0)
            eps_t = sing.tile([1, 1], FP)
            nc.vector.memset(eps_t, EPS)

            # ---- embed: ppT = Wp^T @ xT + bp ----
            xT = sing.tile([2, L], FP)
            nc.sync.dma_start(out=xT, in_=P["xT"][:, :])
            xTb = sing.tile([2, L], BF)
            nc.vector.tensor_copy(out=xTb, in_=xT)
            Wp_t = wload("Wp", 2, DM, tag="wp512x")  # [2, 512] single tile (rows<128)
            bp_t = vec("bp")
            pp_bf = [sing.tile([128, L], BF, tag=f"ppbf{g}", name=f"ppbf{g}") for g in range(NB)]
            for g in range(NB):
                ps = psum.tile([128, L], FP, tag="tr", name="tr")
                nc.tensor.matmul(ps, lhsT=Wp_t[0][:, g * 128:(g + 1) * 128],
                                 rhs=xTb, start=True, stop=True)
                nc.vector.tensor_scalar(out=pp_bf[g], in0=ps, scalar1=bp_t[g],
                                        scalar2=None, op0=OP.add)

            # ---- MHA ----
            def proj_T(wname, bias_ts, outdt=BF):
                """outT[do, t] = W^T @ pp (+bias): returns NB tiles"""
                Wt = wload(wname, DM, DM, tag="w512")
                outs = []
                for m in range(NB):
                    ps = psum.tile([128, L], FP, tag="tr", name="tr")
                    for k in range(NB):
                        nc.tensor.matmul(ps, lhsT=Wt[k][:, m * 128:(m + 1) * 128],
                                         rhs=pp_bf[k], start=(k == 0),
                                         stop=(k == NB - 1))
                    o = sing.tile([128, L], outdt, tag=f"{wname}_o{m}", name=f"{wname}_o{m}")
                    if bias_ts is None:
                        nc.scalar.copy(out=o, in_=ps)
                    else:
                        nc.vector.tensor_scalar(out=o, in0=ps, scalar1=bias_ts[m],
                                                scalar2=None, op0=OP.add)
                    outs.append(o)
                return outs

            qT = proj_T("Wq", vec("bq"))
            kT = proj_T("Wk", vec("bk"))
            # V in natural layout: V[t, d] = pp[t, :] @ Wv
            Wv_t = wload("Wv", DM, DM, tag="w512")
            Vn = []
            for m in range(NB):  # m indexes t-blocks
                ps = psum.tile([128, L], FP, tag="tr", name="tr")
                for k in range(NB):
                    nc.tensor.matmul(ps, lhsT=pp_bf[k][:, m * 128:(m + 1) * 128],
                                     rhs=Wv_t[k], start=(k == 0), stop=(k == NB - 1))
                o = sing.tile([128, L], BF, tag=f"vn{m}", name=f"vn{m}")
                nc.scalar.copy(out=o, in_=ps)
                Vn.append(o)

            oT = [sing.tile([128, L], BF, tag=f"oT{h}", name=f"oT{h}") for h in range(NH)]
            for h in range(NH):
                # ST[m, l] = K_h^T Q_h ; E = exp(ST); denom = ones^T E
                E_h = []
                dn = pss.tile([1, L], FP, tag="sm", name="sm")
                for mb in range(NB):
                    ps = psum.tile([128, L], FP, tag="tr", name="tr")
                    nc.tensor.matmul(ps, lhsT=kT[h][:, mb * 128:(mb + 1) * 128],
                                     rhs=qT[h], start=True, stop=True)
                    e = scr1.tile([128, L], BF, tag=f"eh{mb}", name=f"eh{mb}")
                    nc.scalar.activation(out=e, in_=ps, func=AF.Exp)
                    E_h.append(e)
                ob = scr.tile([1, 128], BF, tag="onesbf", name="onesbf")
                nc.vector.tensor_copy(out=ob, in_=ones_r)
                oc = scr.tile([128, 1], BF, tag="onescbf", name="onescbf")
                nc.vector.tensor_copy(out=oc, in_=ones_c)
                for mb in range(NB):
                    nc.tensor.matmul(dn, lhsT=oc, rhs=E_h[mb],
                                     start=(mb == 0), stop=(mb == NB - 1))
                rinv = scr.tile([1, L], FP, tag="rinv", name="rinv")
                nc.vector.reciprocal_approx_fast(out=rinv, in_=dn)
                rb = scr.tile([1, L], BF, tag="rb", name="rb")
                nc.vector.tensor_copy(out=rb, in_=rinv)
                rrep = psum.tile([128, L], FP, tag="tr", name="tr")
                nc.tensor.matmul(rrep, lhsT=ob, rhs=rb, start=True, stop=True)
                rrs = scr.tile([128, L], FP, tag="rrs", name="rrs")
                nc.scalar.copy(out=rrs, in_=rrep)
                # AV: OT_h = sum_m V[m, dh] E[m, l]
                av = psum.tile([128, L], FP, tag="tr", name="tr")
                for mb in range(NB):
                    nc.tensor.matmul(av, lhsT=Vn[mb][:, h * 128:(h + 1) * 128],
                                     rhs=E_h[mb], start=(mb == 0),
                                     stop=(mb == NB - 1))
                nc.vector.tensor_tensor(out=oT[h], in0=av, in1=rrs, op=OP.mult)

            bo2_t = vec("bo2")
            Wo_t = wload("Wo", DM, DM, tag="w512")
            hT = [sing.tile([128, L], FP, tag=f"hT{g}", name=f"hT{g}") for g in range(NB)]
            for m in range(NB):
                ps = psum.tile([128, L], FP, tag="tr", name="tr")
                for k in range(NB):
                    nc.tensor.matmul(ps, lhsT=Wo_t[k][:, m * 128:(m + 1) * 128],
                                     rhs=oT[k], start=(k == 0), stop=(k == NB - 1))
                nc.vector.tensor_scalar(out=hT[m], in0=ps, scalar1=bo2_t[m],
                                        scalar2=None, op0=OP.add)

            # ---- persistent mamba tiles ----
            NH2 = DS // 4
            dbl_dram = dram.tile([64, L], BF, tag="dbldram", name="dbldram")

            def emit_mamba(li, dd, h_bf, last):
                tg = f"{li}{dd}"
                rev = dd == 1
                Tn = 2 if (last and not rev) else L
                # Win matmuls: x-half always full T (rev) or Tn; z-half Tn2
                def win_half(co):
                    ts = []
                    for k in range(NB):
                        t = wpool.tile([128, DM], BF, tag=f"win_{k}",
                                       name=f"win_{k}")
                        nc.sync.dma_start(
                            out=t, in_=P["Win" + tg][k * 128:(k + 1) * 128,
                                                     co:co + DM])
                        ts.append(t)
                    return ts

                Win_t = win_half(0)
                Tx = L if not last or rev else 3
                xcpre = []
                for m in range(NB):
                    ps = psacc.tile([128, L], FP, tag="acc", name="acc")
                    for k in range(NB):
                        nc.tensor.matmul(ps[:, 0:Tx],
                                         lhsT=Win_t[k][:, m * 128:(m + 1) * 128],
                                         rhs=h_bf[k][:, 0:Tx], start=(k == 0),
                                         stop=(k == NB - 1))
                    xcpre.append(ps)
                Tz = 2 if last else L
                Win_z = win_half(DM)
                zsil = []
                for m in range(NB):
                    ps = psum.tile([128, L], FP, tag="tr", name="tr")
                    for k in range(NB):
                        nc.tensor.matmul(
                            ps[:, 0:Tz],
                            lhsT=Win_z[k][:, m * 128:(m + 1) * 128],
                            rhs=h_bf[k][:, 0:Tz], start=(k == 0), stop=(k == NB - 1))
                    o = sing.tile([128, L], BF, tag=f"zsil{m}", name=f"zsil{m}")
                    nc.scalar.activation(out=o[:, 0:Tz], in_=ps[:, 0:Tz], func=AF.Silu)
                    zsil.append(o)

                convw = P["convw" + tg]
                w0 = [sing.tile([128, 1], FP, tag=f"w0_{g}", name=f"w0_{g}") for g in range(NB)]
                w1 = [sing.tile([128, 1], FP, tag=f"w1_{g}", name=f"w1_{g}") for g in range(NB)]
                for g in range(NB):
                    nc.sync.dma_start(out=w0[g],
                                      in_=convw[g * 128:(g + 1) * 128, 0:1])
                    nc.sync.dma_start(out=w1[g],
                                      in_=convw[g * 128:(g + 1) * 128, 1:2])
                cb_t = vec("convb" + tg)
                xcT = [sing.tile([128, L], BF, tag=f"xcT{g}", name=f"xcT{g}") for g in range(NB)]
                Tc = Tx if (last and not rev) else L
                for g in range(NB):
                    t1 = scr.tile([128, L], FP, tag="convt1", name="convt1")
                    nc.vector.tensor_scalar(out=t1[:, 0:Tc], in0=xcpre[g][:, 0:Tc],
                                            scalar1=w1[g], scalar2=cb_t[g],
                                            op0=OP.mult, op1=OP.add)
                    c2 = scr.tile([128, L], FP, tag="convt2", name="convt2")
                    if not rev:
                        nc.vector.scalar_tensor_tensor(
                            out=c2[:, 1:Tc], in0=xcpre[g][:, 0:Tc - 1],
                            scalar=w0[g], in1=t1[:, 1:Tc], op0=OP.mult, op1=OP.add)
                        nc.vector.tensor_copy(out=c2[:, 0:1], in_=t1[:, 0:1])
                    else:
                        nc.vector.scalar_tensor_tensor(
                            out=c2[:, 0:Tc - 1], in0=xcpre[g][:, 1:Tc],
                            scalar=w0[g], in1=t1[:, 0:Tc - 1], op0=OP.mult,
                            op1=OP.add)
                        nc.vector.tensor_copy(out=c2[:, Tc - 1:Tc],
                                              in_=t1[:, Tc - 1:Tc])
                    nc.scalar.activation(out=xcT[g][:, 0:Tn], in_=c2[:, 0:Tn],
                                         func=AF.Silu)

                # dbl = Wx^T @ xc  [64, Tn]
                Wx_t = wload("Wx" + tg, DM, 64, tag="wx")
                psd = pss.tile([64, L], FP, tag="sm", name="sm")
                for k in range(NB):
                    nc.tensor.matmul(psd[:, 0:Tn], lhsT=Wx_t[k],
                                     rhs=xcT[k][:, 0:Tn],
                                     start=(k == 0), stop=(k == NB - 1))
                dblT = scr.tile([64, L], FP, tag="dblT", name="dblT")
                nc.scalar.copy(out=dblT[:, 0:Tn], in_=psd[:, 0:Tn])
                dbl_bf = scr.tile([64, L], BF, tag="dblbf", name="dblbf")
                nc.vector.tensor_copy(out=dbl_bf[:, 0:Tn], in_=dblT[:, 0:Tn])
                nc.sync.dma_start(out=dbl_dram[:, 0:Tn], in_=dbl_bf[:, 0:Tn])
                dtraw = scr.tile([DTR, L], BF, tag="dtraw", name="dtraw")
                nc.vector.tensor_copy(out=dtraw[:, 0:Tn], in_=dblT[0:DTR, 0:Tn])

                # dt = softplus(Wdt^T @ dtraw + bdt)
                Wdt_t = wload("Wdt" + tg, DTR, DM, tag="wdt512")
                bdt_t = vec("bdt" + tg)
                dtT = [sing.tile([128, L], FP, tag=f"dtT{g}", name=f"dtT{g}") for g in range(NB)]
                duT = [sing.tile([128, L], BF, tag=f"duT{g}", name=f"duT{g}") for g in range(NB)]
                for g in range(NB):
                    ps = psum.tile([128, L], FP, tag="tr", name="tr")
                    nc.tensor.matmul(ps[:, 0:Tn],
                                     lhsT=Wdt_t[0][:, g * 128:(g + 1) * 128],
                                     rhs=dtraw[:, 0:Tn], start=True, stop=True)
                    nc.scalar.activation(out=dtT[g][:, 0:Tn], in_=ps[:, 0:Tn],
                                         func=AF.Exp, bias=bdt_t[g])
                    nc.scalar.activation(out=dtT[g][:, 0:Tn], in_=dtT[g][:, 0:Tn],
                                         func=AF.Ln, bias=1.0)
                    nc.vector.tensor_tensor(out=duT[g][:, 0:Tn],
                                            in0=dtT[g][:, 0:Tn],
                                            in1=xcT[g][:, 0:Tn], op=OP.mult)

                dap = dbl_dram[:, :]
                el = dap.ap[-1][0]

                yT = [sing.tile([128, L], FP, tag=f"yT{g}", name=f"yT{g}") for g in range(NB)]
                small = last and not rev
                yT = None
                yTl = [sing.tile([128, L], FP, tag=f"yT{g}", name=f"yT{g}")
                       for g in range(NB)]
                yt2 = scr.tile([128, L], FP, tag="yt2", name="yt2")
                for nh in range(4):
                    # broadcast B/C halves for this mamba
                    B_rep = bigp.tile([128, NH2, L], BF, tag="Brep",
                                      name="Brep")
                    C_rep = bigp.tile([128, NH2, L], BF, tag="Crep",
                                      name="Crep")
                    def bcast(dst, row0):
                        src = bass.AP(tensor=dap.tensor,
                                      offset=dap.offset + row0 * L * el,
                                      ap=[[0, 128], [L * el, NH2], [el, Tn]])
                        nc.sync.dma_start(out=dst[:, :, 0:Tn], in_=src)
                    bcast(B_rep, DTR + nh * NH2)
                    if not last:
                        bcast(C_rep, DTR + DS + nh * NH2)
                    for g in range(NB):
                        if small:
                            A2s = scr.tile([128, NH2, 2], BF, tag="A2s", name="A2s")
                            dBu2s = scr.tile([128, NH2, 2], BF, tag="dBu2s",
                                             name="dBu2s")
                            At, dBt, Ht2 = A2s, dBu2s, dBu2s
                            AL = 2
                        else:
                            A_blk = bigp.tile([128, NH2, L], BF, tag="Ablk",
                                              name="Ablk")
                            dBu_blk = bigp.tile([128, NH2, L], BF, tag="dBublk",
                                                name="dBublk")
                            At, dBt, Ht2 = A_blk, dBu_blk, dBu_blk
                            AL = L
                        for n in range(NH2):
                            nc.scalar.activation(out=At[:, n, 0:Tn],
                                                 in_=dtT[g][:, 0:Tn], func=AF.Exp,
                                                 scale=-float(nh * NH2 + n + 1))
                        ael = At.ap[-1][0]
                        t0 = 0 if not rev else Tn - 1
                        mask = bass.AP(tensor=At.tensor,
                                       offset=At.offset + t0 * ael,
                                       ap=[At.ap[0], [AL * ael, NH2], [ael, 1]])
                        nc.vector.memset(mask, 0.0)
                        del_ = duT[g].ap[-1][0]
                        du_s0 = bass.AP(tensor=duT[g].tensor, offset=duT[g].offset,
                                        ap=[duT[g].ap[0], [0, NH2], [del_, Tn]])
                        nc.vector.tensor_tensor(out=dBt[:, :, 0:Tn], in0=du_s0,
                                                in1=B_rep[:, :, 0:Tn], op=OP.mult)
                        if not small:
                            if not rev:
                                nc.vector.tensor_tensor_scan(
                                    out=flat2(dBu_blk, NH2 * L),
                                    data0=flat2(A_blk, NH2 * L),
                                    data1=flat2(dBu_blk, NH2 * L), initial=0.0,
                                    op0=OP.mult, op1=OP.add)
                            else:
                                nc.vector.tensor_tensor_scan(
                                    out=rev3(dBu_blk), data0=rev3(A_blk),
                                    data1=rev3(dBu_blk), initial=0.0,
                                    op0=OP.mult, op1=OP.add)
                        else:
                            nc.vector.tensor_tensor_scan(
                                out=flat2(dBu2s, NH2 * 2), data0=flat2(A2s, NH2 * 2),
                                data1=flat2(dBu2s, NH2 * 2), initial=0.0,
                                op0=OP.mult, op1=OP.add)
                        ytarget = yTl[g] if nh == 0 else yt2
                        if not last:
                            ych = Ht2  # in-place: H *= C_rep
                            nc.vector.tensor_tensor(out=ych, in0=Ht2, in1=C_rep,
                                                    op=OP.mult)
                            # n-reduce as bf16 2x add tree over contiguous slices
                            nc.vector.tensor_tensor(out=ych[:, 0, :],
                                                    in0=ych[:, 0, :],
                                                    in1=ych[:, 1, :], op=OP.add)
                            nc.vector.tensor_tensor(out=ych[:, 2, :],
                                                    in0=ych[:, 2, :],
                                                    in1=ych[:, 3, :], op=OP.add)
                            nc.vector.tensor_tensor(out=ytarget, in0=ych[:, 0, :],
                                                    in1=ych[:, 2, :], op=OP.add)
                        else:
                            if small:
                                h_sl = Ht2[:, :, :]
                            else:
                                hel = Ht2.ap[-1][0]
                                h_sl = bass.AP(tensor=Ht2.tensor, offset=Ht2.offset,
                                               ap=[Ht2.ap[0], [L * hel, NH2],
                                                   [hel, 2]])
                            c2t = scr.tile([128, NH2, 2], BF, tag="c2t", name="c2t")
                            csrc = bass.AP(
                                tensor=dap.tensor,
                                offset=dap.offset + (DTR + DS + nh * NH2) * L * el,
                                ap=[[0, 128], [L * el, NH2], [el, 2]])
                            nc.sync.dma_start(out=c2t, in_=csrc)
                            tmp = scr.tile([128, NH2, 2], BF, tag="ychs",
                                           name="ychs")
                            nc.vector.tensor_tensor(out=tmp, in0=h_sl, in1=c2t,
                                                    op=OP.mult)
                            tel = tmp.ap[-1][0]
                            red_in = bass.AP(tensor=tmp.tensor, offset=tmp.offset,
                                             ap=[tmp.ap[0], [tel, 2],
                                                 [2 * tel, NH2]])
                            nc.vector.tensor_reduce(out=ytarget[:, 0:2],
                                                    in_=red_in,
                                                    axis=mybir.AxisListType.X,
                                                    op=OP.add)
                        if nh > 0:
                            Ty = 2 if last else L
                            nc.vector.tensor_tensor(out=yTl[g][:, 0:Ty],
                                                    in0=yTl[g][:, 0:Ty],
                                                    in1=yt2[:, 0:Ty], op=OP.add)
                yT = yTl

                # gate: g = (y + xc) * zsil  -> bf16
                gT = [scr.tile([128, L], BF, tag=f"gT{g}", name=f"gT{g}") for g in range(NB)]
                Tg = 2 if last else L
                for g in range(NB):
                    nc.vector.tensor_tensor(out=yT[g][:, 0:Tg], in0=yT[g][:, 0:Tg],
                                            in1=xcT[g][:, 0:Tg], op=OP.add)
                    nc.vector.tensor_tensor(out=gT[g][:, 0:Tg], in0=yT[g][:, 0:Tg],
                                            in1=zsil[g][:, 0:Tg], op=OP.mult)
                return gT, Tg

            def emit_layer(li):
                last = li == 1
                h_bf = [scr1.tile([128, L], BF, tag=f"hbf{g}", name=f"hbf{g}") for g in range(NB)]
                for g in range(NB):
                    nc.vector.tensor_copy(out=h_bf[g], in_=hT[g])
                g_f, Tg_f = emit_mamba(li, 0, h_bf, last)
                g_r, Tg_r = emit_mamba(li, 1, h_bf, last)
                Tm = 2 if last else L
                pso = [psacc.tile([128, L], FP, tag="acc", name="acc")
                       for _ in range(NB)]
                for dd, gg in ((0, g_f), (1, g_r)):
                    Wd = wload(f"Wout{li}{dd}", DM, DM, tag="wout")
                    for m in range(NB):
                        for k in range(NB):
                            nc.tensor.matmul(
                                pso[m][:, 0:Tm],
                                lhsT=Wd[k][:, m * 128:(m + 1) * 128],
                                rhs=gg[k][:, 0:Tm], start=(dd == 0 and k == 0),
                                stop=(dd == 1 and k == NB - 1))
                for m in range(NB):
                    nc.vector.tensor_tensor(out=hT[m][:, 0:Tm],
                                            in0=hT[m][:, 0:Tm], in1=pso[m][:, 0:Tm],
                                            op=OP.add)
                ln_inplace(Tm)
                ffn(li, Tm, last)

            def ln_inplace(T):
                """layernorm over d (partitions) of hT[:, 0:T], in place."""
                psm = pss.tile([1, L], FP, tag="sm", name="sm")
                psq = pss.tile([1, L], FP, tag="sm", name="sm")
                for g in range(NB):
                    sq = scr.tile([128, L], FP, tag="lntmp", name="lntmp")
                    nc.scalar.activation(out=sq[:, 0:T], in_=hT[g][:, 0:T],
                                         func=AF.Square)
                    nc.tensor.matmul(psm[:, 0:T], lhsT=ones_c, rhs=hT[g][:, 0:T],
                                     start=(g == 0), stop=(g == NB - 1))
                    nc.tensor.matmul(psq[:, 0:T], lhsT=ones_c, rhs=sq[:, 0:T],
                                     start=(g == 0), stop=(g == NB - 1))
                mean = scr.tile([1, L], FP, tag="lnmean", name="lnmean")
                nc.vector.tensor_scalar(out=mean[:, 0:T], in0=psm[:, 0:T],
                                        scalar1=1.0 / DM, scalar2=None, op0=OP.mult)
                m2 = scr.tile([1, L], FP, tag="lnm2", name="lnm2")
                nc.vector.tensor_tensor(out=m2[:, 0:T], in0=mean[:, 0:T],
                                        in1=mean[:, 0:T], op=OP.mult)
                var = scr.tile([1, L], FP, tag="lnvar", name="lnvar")
                nc.vector.scalar_tensor_tensor(out=var[:, 0:T], in0=psq[:, 0:T],
                                               scalar=1.0 / DM, in1=m2[:, 0:T],
                                               op0=OP.mult, op1=OP.subtract)
                sd = scr.tile([1, L], FP, tag="lnsd", name="lnsd")
                nc.scalar.activation(out=sd[:, 0:T], in_=var[:, 0:T],
                                     func=AF.Sqrt, bias=eps_t)
                rinv = scr.tile([1, L], FP, tag="lnrinv", name="lnrinv")
                nc.vector.reciprocal_approx_fast(out=rinv[:, 0:T], in_=sd[:, 0:T])
                mrep = psum.tile([128, L], FP, tag="tr", name="tr")
                nc.tensor.matmul(mrep[:, 0:T], lhsT=ones_r, rhs=mean[:, 0:T],
                                 start=True, stop=True)
                rrep = psum.tile([128, L], FP, tag="tr", name="tr")
                nc.tensor.matmul(rrep[:, 0:T], lhsT=ones_r, rhs=rinv[:, 0:T],
                                 start=True, stop=True)
                mrs = scr.tile([128, L], FP, tag="lnmrs", name="lnmrs")
                nc.scalar.copy(out=mrs[:, 0:T], in_=mrep[:, 0:T])
                rrs = scr.tile([128, L], FP, tag="lnrrs", name="lnrrs")
                nc.scalar.copy(out=rrs[:, 0:T], in_=rrep[:, 0:T])
                for g in range(NB):
                    c = scr.tile([128, L], FP, tag="lntmp", name="lntmp")
                    nc.vector.tensor_tensor(out=c[:, 0:T], in0=hT[g][:, 0:T],
                                            in1=mrs[:, 0:T], op=OP.subtract)
                    nc.vector.tensor_tensor(out=hT[g][:, 0:T], in0=c[:, 0:T],
                                            in1=rrs[:, 0:T], op=OP.mult)

            def ffn(li, T, last):
                h_bf = [scr1.tile([128, L], BF, tag=f"fhbf{g}", name=f"fhbf{g}") for g in range(NB)]
                for g in range(NB):
                    nc.vector.tensor_copy(out=h_bf[g][:, 0:T], in_=hT[g][:, 0:T])
                b1 = vec(f"ffb1_{li}", DF)
                b2 = vec(f"ffb2_{li}")
                pso = [psacc.tile([128, L], FP, tag="acc", name="acc")
                       for _ in range(NB)]
                for half in range(4):
                    W1 = []
                    for k in range(NB):
                        t = wpool.tile([128, DF // 4], BF, tag=f"ffw1_{k}",
                                       name=f"ffw1_{k}")
                        nc.sync.dma_start(
                            out=t, in_=P[f"ffW1_{li}"][k * 128:(k + 1) * 128,
                                                       half * (DF // 4):
                                                       (half + 1) * (DF // 4)])
                        W1.append(t)
                    yb = [scr1.tile([128, L], BF, tag=f"ffyb{k}", name=f"ffyb{k}")
                          for k in range(4)]
                    for k8 in range(4):
                        m = half * 4 + k8
                        ps = psum.tile([128, L], FP, tag="tr", name="tr")
                        for k in range(NB):
                            nc.tensor.matmul(ps[:, 0:T],
                                             lhsT=W1[k][:, k8 * 128:(k8 + 1) * 128],
                                             rhs=h_bf[k][:, 0:T], start=(k == 0),
                                             stop=(k == NB - 1))
                        nc.scalar.activation(out=yb[k8][:, 0:T], in_=ps[:, 0:T],
                                             func=AF.Relu, bias=b1[m])
                    W2h = []
                    for k8 in range(4):
                        t = wpool.tile([128, DM], BF, tag=f"ffw2_{k8}",
                                       name=f"ffw2_{k8}")
                        r0 = (half * 4 + k8) * 128
                        nc.sync.dma_start(out=t,
                                          in_=P[f"ffW2_{li}"][r0:r0 + 128, :])
                        W2h.append(t)
                    for m in range(NB):
                        for k8 in range(4):
                            nc.tensor.matmul(
                                pso[m][:, 0:T],
                                lhsT=W2h[k8][:, m * 128:(m + 1) * 128],
                                rhs=yb[k8][:, 0:T], start=(half == 0 and k8 == 0),
                                stop=(half == 3 and k8 == 3))
                for m in range(NB):
                    nc.vector.scalar_tensor_tensor(out=hT[m][:, 0:T],
                                                   in0=pso[m][:, 0:T], scalar=b2[m],
                                                   in1=hT[m][:, 0:T], op0=OP.add,
                                                   op1=OP.add)
                ln_inplace(T)

            emit_layer(0)
            emit_layer(1)

            # final projection at positions 0,1
            h_bf = [scr.tile([128, 2], BF, tag=f"pjb{g}", name=f"pjb{g}") for g in range(NB)]
            for g in range(NB):
                nc.vector.tensor_copy(out=h_bf[g], in_=hT[g][:, 0:2])
            PW = wload("projW", DM, PRED, tag="w512")
            pb = sing.tile([PRED, 1], FP)
            nc.sync.dma_start(out=pb, in_=P["projb"][:])
            ps = pss.tile([PRED, 2], FP, tag="sm", name="sm")
            for k in range(NB):
                nc.tensor.matmul(ps, lhsT=PW[k], rhs=h_bf[k], start=(k == 0),
                                 stop=(k == NB - 1))
            res = sing.tile([PRED, 2], FP)
            nc.vector.tensor_scalar(out=res, in0=ps, scalar1=pb, scalar2=None,
                                    op0=OP.add)
            nc.sync.dma_start(out=out_d[:, :], in_=res)

    nc.finalize()
    return nc


_CACHE = {}


def kernel(**inputs):
    w, xts, means, stdev = prep_host_inputs(inputs)
    if "nc" not in _CACHE:
        _CACHE["nc"] = build_program()
    nc = _CACHE["nc"]
    in_maps = []
    for b in range(8):
        m = dict(w)
        m["xT"] = xts[b]
        in_maps.append(m)
    rr = run_bass_kernel_spmd(nc, in_maps, list(range(8)))
    outs = []
    for b in range(8):
        o = np.asarray(rr.results[b]["out"], np.float32)     # [96, 2]
        o = o * stdev[b][None, :] + means[b][None, :]
        outs.append(o)
    return np.stack(outs)                                    # [8, 96, 2]
